# revision 6
# baseline (speedup 1.0000x reference)
"""Trainium2 Bass kernel for nn_LlamaMoDDecoderLayer (MoD decoder layer).

Strategy (8 NeuronCores, tensor-parallel, feature-major layouts):
  - All activations transposed: X^T [feature, token]; every matmul contracts
    over partitions with no activation transposes.
  - Attention: heads sharded 2/core; transposed-scores causal softmax with
    denominator via ones-matmul; per-core head context AllGathered (bf16);
    Wo column-sharded.
  - hs2 = hs + mask_attn*attn computed per-core on its 256 rows (fp32 kept
    for the final residual), AllGathered bf16 for replicated RMSNorm2.
  - MLP: w_gate/w_up column-sharded, w_down row-sharded; bf16 partial
    outputs summed by ReduceScatter so each core ends with its 256 rows.
  - Router argmax masks in exact fp32 (GPSIMD MAC chain + fp32 PE reduce).
  - Matmuls bf16 (host-cast weights), fp32 PSUM accumulation.
"""

import numpy as np
import ml_dtypes

import concourse.bass as bass
import concourse.bacc as bacc
import concourse.mybir as mybir
import concourse.tile as tile
from concourse.alu_op_type import AluOpType
from concourse.bass_utils import run_bass_kernel_spmd

F32 = mybir.dt.float32
BF16 = mybir.dt.bfloat16
AF = mybir.ActivationFunctionType

S, D, H, Dh, F = 2048, 2048, 16, 128, 8192
NC = 8
HPC = H // NC            # heads per core (2)
DCC = D // NC            # output cols per core (256)
FPC = F // NC            # mlp hidden per core (1024)
NDT = D // 128           # 16 d-tiles
NFT = FPC // 128         # 8 local f-tiles
NSC = S // 512           # 4 s-chunks of 512
EPS = 1e-5
THETA = 10000.0

_CACHE = {}

def _build_program():
    nc = bacc.Bacc("TRN2", target_bir_lowering=False, debug=False,
                   num_devices=NC)
    rg = [list(range(NC))]

    d_hsT = nc.dram_tensor("hsT", [D, S], F32, kind="ExternalInput")
    d_hres = nc.dram_tensor("hres", [DCC, S], F32, kind="ExternalInput")
    d_wq = nc.dram_tensor("wq", [D, DCC], BF16, kind="ExternalInput")
    d_wk = nc.dram_tensor("wk", [D, DCC], BF16, kind="ExternalInput")
    d_wv = nc.dram_tensor("wv", [D, DCC], BF16, kind="ExternalInput")
    d_wo = nc.dram_tensor("wo", [D, DCC], BF16, kind="ExternalInput")
    d_wg = nc.dram_tensor("wg", [D, FPC], BF16, kind="ExternalInput")
    d_wu = nc.dram_tensor("wu", [D, FPC], BF16, kind="ExternalInput")
    d_wd = nc.dram_tensor("wd", [FPC, D], BF16, kind="ExternalInput")
    d_qcos = nc.dram_tensor("qcos", [Dh, S], BF16, kind="ExternalInput")
    d_qsin = nc.dram_tensor("qsin", [Dh, S], BF16, kind="ExternalInput")
    d_kcos = nc.dram_tensor("kcos", [Dh, S], BF16, kind="ExternalInput")
    d_ksin = nc.dram_tensor("ksin", [Dh, S], BF16, kind="ExternalInput")
    d_tri = nc.dram_tensor("tri", [128, 4 * 512], BF16, kind="ExternalInput")
    d_rd = nc.dram_tensor("rd", [128, 2 * NDT], F32, kind="ExternalInput")
    d_thr = nc.dram_tensor("thr", [1, 2], F32, kind="ExternalInput")
    d_out = nc.dram_tensor("out", [DCC, S], F32, kind="ExternalOutput")
    d_dbgq = nc.dram_tensor("dbgq", [DCC, S], BF16, kind="ExternalOutput")
    d_dbgc = nc.dram_tensor("dbgc", [DCC, S], BF16, kind="ExternalOutput")
    d_dbgh = nc.dram_tensor("dbgh", [DCC, S], BF16, kind="ExternalOutput")
    d_dbgx = nc.dram_tensor("dbgx", [128, S], BF16, kind="ExternalOutput")
    d_dbgt = nc.dram_tensor("dbgt", [128, S], BF16, kind="ExternalOutput")
    d_dbgp = nc.dram_tensor("dbgp", [128, S], BF16, kind="ExternalOutput")
    d_dbgr = nc.dram_tensor("dbgr", [DCC, S], BF16, kind="ExternalOutput")

    cc1_in = nc.dram_tensor("cc1_in", [DCC, S], BF16)
    cc1_out = nc.dram_tensor("cc1_out", [D, S], BF16, addr_space="Shared")
    cc2_in = nc.dram_tensor("cc2_in", [DCC, S], BF16)
    cc2_out = nc.dram_tensor("cc2_out", [D, S], BF16, addr_space="Shared")
    cc3_in = nc.dram_tensor("cc3_in", [D, S], BF16)
    cc3_out = nc.dram_tensor("cc3_out", [DCC, S], BF16)

    hsT_t = d_hsT.ap().rearrange("(a p) s -> p a s", p=128)
    hres_t = d_hres.ap().rearrange("(a p) s -> p a s", p=128)
    wq_t = d_wq.ap().rearrange("(a p) m -> p a m", p=128)
    wk_t = d_wk.ap().rearrange("(a p) m -> p a m", p=128)
    wv_t = d_wv.ap().rearrange("(a p) m -> p a m", p=128)
    wo_t = d_wo.ap().rearrange("(a p) m -> p a m", p=128)
    wg_t = d_wg.ap().rearrange("(a p) m -> p a m", p=128)
    wu_t = d_wu.ap().rearrange("(a p) m -> p a m", p=128)
    wd_t = d_wd.ap().rearrange("(a p) m -> p a m", p=128)
    cc1i_t = cc1_in.ap().rearrange("(a p) s -> p a s", p=128)
    cc2i_t = cc2_in.ap().rearrange("(a p) s -> p a s", p=128)
    cc3i_t = cc3_in.ap().rearrange("(a p) s -> p a s", p=128)
    cc1o_t = cc1_out.ap().rearrange("(a p) s -> p a s", p=128)
    cc2o_t = cc2_out.ap().rearrange("(a p) s -> p a s", p=128)
    cc3o_t = cc3_out.ap().rearrange("(a p) s -> p a s", p=128)
    out_t = d_out.ap().rearrange("(a p) s -> p a s", p=128)
    dbgq_t = d_dbgq.ap().rearrange("(a p) s -> p a s", p=128)
    dbgc_t = d_dbgc.ap().rearrange("(a p) s -> p a s", p=128)
    dbgh_t = d_dbgh.ap().rearrange("(a p) s -> p a s", p=128)
    dbgr_t = d_dbgr.ap().rearrange("(a p) s -> p a s", p=128)

    with tile.TileContext(nc) as tc:
        with (
            tc.tile_pool(name="const", bufs=1) as cst,
            tc.tile_pool(name="masks", bufs=1) as mkp,
            tc.tile_pool(name="psum", bufs=2, space="PSUM") as psp,
        ):
            ones_b = cst.tile([128, 1], BF16)
            nc.gpsimd.memset(ones_b[:], 1.0)
            ones_r = cst.tile([1, 128], F32)
            nc.gpsimd.memset(ones_r[:], 1.0)
            ones_f = cst.tile([128, 1], F32)
            nc.gpsimd.memset(ones_f[:], 1.0)
            eps1 = cst.tile([1, 1], F32)
            nc.gpsimd.memset(eps1[:], EPS)
            rd = cst.tile([128, 2 * NDT], F32, name="rd")
            nc.sync.dma_start(rd[:], d_rd.ap())
            thr = cst.tile([1, 2], F32, name="thr")
            nc.sync.dma_start(thr[:], d_thr.ap())
            ma_b = mkp.tile([128, S], F32, name="ma_b")
            mm_b = mkp.tile([128, S], F32, name="mm_b")

            with (
                tc.tile_pool(name="attnconst", bufs=1) as acst,
                tc.tile_pool(name="xn", bufs=1) as xnp,
            ):
                qcos = acst.tile([128, S], BF16, name="qcos")
                qsin = acst.tile([128, S], BF16, name="qsin")
                kcos = acst.tile([128, S], BF16, name="kcos")
                ksin = acst.tile([128, S], BF16, name="ksin")
                nc.sync.dma_start(qcos[:], d_qcos.ap())
                nc.sync.dma_start(qsin[:], d_qsin.ap())
                nc.sync.dma_start(kcos[:], d_kcos.ap())
                nc.sync.dma_start(ksin[:], d_ksin.ap())
                tri = acst.tile([128, 4, 512], BF16, name="tri")
                nc.sync.dma_start(
                    tri[:], d_tri.ap().rearrange("p (a m) -> p a m", m=512))
                xnT = xnp.tile([128, NDT, S], BF16, name="xnT")

                # ---- phase 1: stream hsT twice; routers; norm1; xnT ----
                with tc.tile_pool(name="ph1", bufs=1) as p1:
                    dacc_a = p1.tile([128, S], F32, name="dacc_a")
                    dacc_m = p1.tile([128, S], F32, name="dacc_m")
                    acc = p1.tile([128, S], F32, name="acc")
                    r1b = p1.tile([128, S], F32, name="r1b")
                    r1row = p1.tile([1, S], F32, name="r1row")
                    for a in range(NDT):
                        ht = p1.tile([128, S], F32, tag="hst", bufs=3)
                        nc.sync.dma_start(ht[:], hsT_t[:, a, :])
                        sqt = p1.tile([128, S], BF16, tag="sq", bufs=3)
                        nc.scalar.activation(sqt[:], ht[:], AF.Square)
                        if a == 0:
                            nc.vector.tensor_copy(acc[:], sqt[:])
                        else:
                            nc.vector.tensor_tensor(acc[:], acc[:], sqt[:],
                                                    op=AluOpType.add)
                        if a == 0:
                            nc.vector.tensor_scalar(
                                dacc_a[:], ht[:], rd[:, 0:1], None,
                                op0=AluOpType.mult)
                            nc.vector.tensor_scalar(
                                dacc_m[:], ht[:], rd[:, NDT:NDT + 1], None,
                                op0=AluOpType.mult)
                        else:
                            nc.vector.scalar_tensor_tensor(
                                dacc_a[:], ht[:], rd[:, a:a + 1], dacc_a[:],
                                op0=AluOpType.mult, op1=AluOpType.add)
                            nc.vector.scalar_tensor_tensor(
                                dacc_m[:], ht[:], rd[:, NDT + a:NDT + a + 1],
                                dacc_m[:], op0=AluOpType.mult,
                                op1=AluOpType.add)
                    for sc in range(NSC):
                        rp = psp.tile([1, 512], F32, tag="rowps")
                        nc.tensor.matmul(rp[:], ones_f[:],
                                         acc[:, bass.ts(sc, 512)])
                        nc.scalar.activation(r1row[:, bass.ts(sc, 512)], rp[:],
                                             AF.Sqrt, bias=eps1[:],
                                             scale=1.0 / D)
                        nc.vector.reciprocal(r1row[:, bass.ts(sc, 512)],
                                             r1row[:, bass.ts(sc, 512)])
                        bcp = psp.tile([128, 512], F32, tag="mmps")
                        nc.tensor.matmul(bcp[:], ones_r[:],
                                         r1row[:, bass.ts(sc, 512)])
                        nc.scalar.copy(r1b[:, bass.ts(sc, 512)], bcp[:])
                    for a in range(NDT):
                        ht2 = p1.tile([128, S], F32, tag="hst2", bufs=2)
                        nc.sync.dma_start(ht2[:], hsT_t[:, a, :])
                        nc.vector.tensor_tensor(xnT[:, a, :], ht2[:], r1b[:],
                                                op=AluOpType.mult)
                    for dacc, ti, mb in (
                        (dacc_a, 0, ma_b),
                        (dacc_m, 1, mm_b),
                    ):
                        for sc in range(NSC):
                            dps = psp.tile([1, 512], F32, tag="rowps")
                            nc.tensor.matmul(dps[:], ones_f[:],
                                             dacc[:, bass.ts(sc, 512)])
                            mrow = p1.tile([1, 512], F32, tag="mrow", bufs=2)
                            nc.vector.tensor_scalar(
                                mrow[:], dps[:],
                                thr[:, ti:ti + 1], None, op0=AluOpType.is_le)
                            nc.gpsimd.partition_broadcast(
                                mb[:, bass.ts(sc, 512)], mrow[:])

                # ---- phase 2: QKV + rope; phase 3: attention ----
                with tc.tile_pool(name="qkv", bufs=1) as qkp:
                    wq = qkp.tile([128, NDT, DCC], BF16, name="wq")
                    wk = qkp.tile([128, NDT, DCC], BF16, name="wk")
                    wv = qkp.tile([128, NDT, DCC], BF16, name="wv")
                    nc.sync.dma_start(wq[:], wq_t)
                    nc.sync.dma_start(wk[:], wk_t)
                    nc.sync.dma_start(wv[:], wv_t)
                    q_sb = qkp.tile([128, HPC, S], BF16, name="q_sb")
                    k_sb = qkp.tile([128, HPC, S], BF16, name="k_sb")
                    qs_sb = qkp.tile([128, HPC, S], BF16, name="qs_sb")
                    ks_sb = qkp.tile([128, HPC, S], BF16, name="ks_sb")
                    for w_sb, t_sb in ((wq, q_sb), (wk, k_sb)):
                        for mc in range(HPC):
                            for sc in range(NSC):
                                ps = psp.tile([128, 512], F32, tag="mmps")
                                for a in range(NDT):
                                    nc.tensor.matmul(
                                        ps[:], w_sb[:, a, bass.ts(mc, 128)],
                                        xnT[:, a, bass.ts(sc, 512)],
                                        start=(a == 0), stop=(a == NDT - 1))
                                nc.scalar.copy(t_sb[:, mc, bass.ts(sc, 512)],
                                               ps[:])
                    for mc in range(HPC):
                        nc.sync.dma_start(dbgq_t[:, mc, :], q_sb[:, mc, :])
                    for src, dst in ((q_sb, qs_sb), (k_sb, ks_sb)):
                        for mc in range(HPC):
                            nc.sync.dma_start(dst[0:64, mc, :],
                                              src[64:128, mc, :])
                            nc.sync.dma_start(dst[64:128, mc, :],
                                              src[0:64, mc, :])
                    qr = qkp.tile([128, HPC, S], BF16, name="qr")
                    kr = qkp.tile([128, HPC, S], BF16, name="kr")
                    for mc in range(HPC):
                        tq = qkp.tile([128, S], BF16, tag="ropetmp", bufs=2)
                        nc.vector.tensor_tensor(tq[:], qs_sb[:, mc, :],
                                                qsin[:], op=AluOpType.mult)
                        nc.vector.tensor_tensor(qr[:, mc, :], q_sb[:, mc, :],
                                                qcos[:], op=AluOpType.mult)
                        nc.vector.tensor_tensor(qr[:, mc, :], qr[:, mc, :],
                                                tq[:], op=AluOpType.add)
                        tk = qkp.tile([128, S], BF16, tag="ropetmp", bufs=2)
                        nc.vector.tensor_tensor(tk[:], ks_sb[:, mc, :],
                                                ksin[:], op=AluOpType.mult)
                        nc.vector.tensor_tensor(kr[:, mc, :], k_sb[:, mc, :],
                                                kcos[:], op=AluOpType.mult)
                        nc.vector.tensor_tensor(kr[:, mc, :], kr[:, mc, :],
                                                tk[:], op=AluOpType.add)
                    v_sb = qkp.tile([128, NDT, DCC], BF16, name="v_sb")
                    for mc in range(NDT):
                        ps = psp.tile([128, DCC], F32, tag="mmps")
                        for a in range(NDT):
                            nc.tensor.matmul(ps[:],
                                             xnT[:, a, bass.ts(mc, 128)],
                                             wv[:, a, :],
                                             start=(a == 0),
                                             stop=(a == NDT - 1))
                        nc.scalar.copy(v_sb[:, mc, :], ps[:])

                    ctxT = qkp.tile([128, HPC, S], BF16, name="ctxT")
                    for h in range(HPC):
                        for qc in range(NSC):
                            nkt = 4 * (qc + 1)
                            cps = psp.tile([128, 512], F32, tag="ctxps",
                                           bufs=1)
                            dps = psp.tile([1, 512], F32, tag="rowps")
                            for kt in range(nkt):
                                sps = psp.tile([128, 512], F32, tag="stps")
                                nc.tensor.matmul(sps[:],
                                                 kr[:, h, bass.ts(kt, 128)],
                                                 qr[:, h, bass.ts(qc, 512)])
                                est = qkp.tile([128, 512], BF16, tag="est",
                                               bufs=3)
                                nc.scalar.activation(est[:], sps[:], AF.Exp)
                                if kt // 4 == qc:
                                    nc.vector.tensor_tensor(
                                        est[:], est[:], tri[:, kt % 4, :],
                                        op=AluOpType.mult)
                                nc.tensor.matmul(cps[:],
                                                 v_sb[:, kt, bass.ts(h, 128)],
                                                 est[:], start=(kt == 0),
                                                 stop=(kt == nkt - 1))
                                nc.tensor.matmul(dps[:], ones_b[:], est[:],
                                                 start=(kt == 0),
                                                 stop=(kt == nkt - 1))
                            rrow = qkp.tile([1, 512], F32, tag="rrow", bufs=1)
                            nc.vector.reciprocal(rrow[:], dps[:])
                            rb = qkp.tile([128, 512], F32, tag="rb", bufs=2)
                            nc.gpsimd.partition_broadcast(rb[:], rrow[:])
                            nc.vector.tensor_tensor(
                                ctxT[:, h, bass.ts(qc, 512)], cps[:], rb[:],
                                op=AluOpType.mult)
                    for mc in range(HPC):
                        nc.sync.dma_start(cc1i_t[:, mc, :], ctxT[:, mc, :])
                        nc.sync.dma_start(dbgc_t[:, mc, :], ctxT[:, mc, :])

            # ---- phase 4: AG ctx + Wo proj + hs2 ----
            nc.gpsimd.collective_compute(
                "AllGather", AluOpType.bypass, replica_groups=rg,
                ins=[cc1_in.ap()], outs=[cc1_out.ap()])
            with tc.tile_pool(name="p46", bufs=1) as p46:
                hres = p46.tile([128, 2, S], F32, name="hres")
                nc.sync.dma_start(hres[:], hres_t)
                hs2f = p46.tile([128, 2, S], F32, name="hs2f")
                with tc.tile_pool(name="wo_ph", bufs=1) as wop:
                    ctxg = wop.tile([128, NDT, S], BF16, name="ctxg")
                    for a in range(NDT):
                        nc.sync.dma_start(ctxg[:, a, :], cc1o_t[:, a, :])
                    wo = wop.tile([128, NDT, DCC], BF16, name="wo")
                    nc.sync.dma_start(wo[:], wo_t)
                    hs2b = wop.tile([128, 2, S], BF16, name="hs2b")
                    for mc in range(HPC):
                        for sc in range(NSC):
                            ps = psp.tile([128, 512], F32, tag="mmps")
                            for a in range(NDT):
                                nc.tensor.matmul(
                                    ps[:], wo[:, a, bass.ts(mc, 128)],
                                    ctxg[:, a, bass.ts(sc, 512)],
                                    start=(a == 0), stop=(a == NDT - 1))
                            t = wop.tile([128, 512], F32, tag="wot", bufs=2)
                            nc.vector.tensor_tensor(
                                t[:], ps[:], ma_b[:, bass.ts(sc, 512)],
                                op=AluOpType.mult)
                            nc.vector.tensor_tensor(
                                hs2f[:, mc, bass.ts(sc, 512)], t[:],
                                hres[:, mc, bass.ts(sc, 512)],
                                op=AluOpType.add)
                            nc.scalar.copy(hs2b[:, mc, bass.ts(sc, 512)],
                                           hs2f[:, mc, bass.ts(sc, 512)])
                    for mc in range(HPC):
                        nc.sync.dma_start(cc2i_t[:, mc, :], hs2b[:, mc, :])
                        nc.sync.dma_start(dbgh_t[:, mc, :], hs2b[:, mc, :])
                nc.gpsimd.collective_compute(
                    "AllGather", AluOpType.bypass, replica_groups=rg,
                    ins=[cc2_in.ap()], outs=[cc2_out.ap()])

                # ---- phase 5: norm2 + MLP ----
                with tc.tile_pool(name="mlp", bufs=1) as mlp:
                    hs2g = mlp.tile([128, NDT, S], BF16, name="hs2g")
                    for a in range(NDT):
                        nc.sync.dma_start(hs2g[:, a, :], cc2o_t[:, a, :])
                    with tc.tile_pool(name="r2p", bufs=1) as r2p:
                        r2row = r2p.tile([1, S], F32, name="r2row")
                        r2b = r2p.tile([128, S], F32, name="r2b")
                        for sc in range(NSC):
                            ssp = psp.tile([1, 512], F32, tag="rowps")
                            for a in range(NDT):
                                sqt = r2p.tile([128, 512], BF16, tag="sq2",
                                               bufs=3)
                                nc.scalar.activation(
                                    sqt[:], hs2g[:, a, bass.ts(sc, 512)],
                                    AF.Square)
                                nc.tensor.matmul(ssp[:], ones_b[:], sqt[:],
                                                 start=(a == 0),
                                                 stop=(a == NDT - 1))
                            nc.scalar.activation(r2row[:, bass.ts(sc, 512)],
                                                 ssp[:], AF.Sqrt,
                                                 bias=eps1[:], scale=1.0 / D)
                            nc.vector.reciprocal(r2row[:, bass.ts(sc, 512)],
                                                 r2row[:, bass.ts(sc, 512)])
                            bcp = psp.tile([128, 512], F32, tag="mmps")
                            nc.tensor.matmul(bcp[:], ones_r[:],
                                             r2row[:, bass.ts(sc, 512)])
                            nc.scalar.copy(r2b[:, bass.ts(sc, 512)], bcp[:])
                        for a in range(NDT):
                            nc.vector.tensor_tensor(
                                hs2g[:, a, :], hs2g[:, a, :], r2b[:],
                                op=AluOpType.mult)
                    xn2 = hs2g  # normalized in place
                    nc.sync.dma_start(d_dbgx.ap(), xn2[:, 0, :])
                    hT = mlp.tile([128, NFT, S], BF16, name="hT")
                    with tc.tile_pool(name="wstream", bufs=3) as wsp:
                        for fc in range(NFT):
                            wgc = wsp.tile([128, NDT, 128], BF16, tag="wgc")
                            nc.sync.dma_start(wgc[:],
                                              wg_t[:, :, bass.ts(fc, 128)])
                            sg = wsp.tile([128, S], BF16, tag="sg", bufs=2)
                            for sc in range(NSC):
                                ps = psp.tile([128, 512], F32, tag="mmps")
                                for a in range(NDT):
                                    nc.tensor.matmul(
                                        ps[:], wgc[:, a, :],
                                        xn2[:, a, bass.ts(sc, 512)],
                                        start=(a == 0), stop=(a == NDT - 1))
                                nc.scalar.activation(sg[:, bass.ts(sc, 512)],
                                                     ps[:], AF.Silu)
                            wuc = wsp.tile([128, NDT, 128], BF16, tag="wuc")
                            nc.sync.dma_start(wuc[:],
                                              wu_t[:, :, bass.ts(fc, 128)])
                            for sc in range(NSC):
                                ps = psp.tile([128, 512], F32, tag="mmps")
                                for a in range(NDT):
                                    nc.tensor.matmul(
                                        ps[:], wuc[:, a, :],
                                        xn2[:, a, bass.ts(sc, 512)],
                                        start=(a == 0), stop=(a == NDT - 1))
                                nc.vector.tensor_tensor(
                                    hT[:, fc, bass.ts(sc, 512)], ps[:],
                                    sg[:, bass.ts(sc, 512)],
                                    op=AluOpType.mult)
                        for mc in range(NDT):
                            wdc = wsp.tile([128, NFT, 128], BF16, tag="wdc")
                            nc.sync.dma_start(wdc[:],
                                              wd_t[:, :, bass.ts(mc, 128)])
                            for sc in range(NSC):
                                ps = psp.tile([128, 512], F32, tag="mmps")
                                for a in range(NFT):
                                    nc.tensor.matmul(
                                        ps[:], wdc[:, a, :],
                                        hT[:, a, bass.ts(sc, 512)],
                                        start=(a == 0), stop=(a == NFT - 1))
                                stg = wsp.tile([128, 512], BF16, tag="stg",
                                               bufs=3)
                                nc.scalar.copy(stg[:], ps[:])
                                nc.sync.dma_start(
                                    cc3i_t[:, mc, bass.ts(sc, 512)], stg[:])
                                if mc == 0:
                                    nc.sync.dma_start(
                                        d_dbgp.ap()[:, bass.ts(sc, 512)],
                                        stg[:])
                        nc.sync.dma_start(d_dbgt.ap(), hT[:, 0, :])
                nc.gpsimd.collective_compute(
                    "ReduceScatter", AluOpType.add, replica_groups=rg,
                    ins=[cc3_in.ap()], outs=[cc3_out.ap()])

                # ---- phase 6: final residual ----
                with tc.tile_pool(name="fin", bufs=1) as fin:
                    rs = fin.tile([128, 2, S], BF16, name="rs")
                    for mc in range(HPC):
                        nc.sync.dma_start(rs[:, mc, :], cc3o_t[:, mc, :])
                        nc.sync.dma_start(dbgr_t[:, mc, :], rs[:, mc, :])
                    outt = fin.tile([128, 2, S], F32, name="outt")
                    for mc in range(HPC):
                        t2 = fin.tile([128, S], F32, tag="fint", bufs=2)
                        nc.vector.tensor_tensor(t2[:], rs[:, mc, :], mm_b[:],
                                                op=AluOpType.mult)
                        nc.vector.tensor_tensor(outt[:, mc, :], t2[:],
                                                hs2f[:, mc, :],
                                                op=AluOpType.add)
                        nc.sync.dma_start(out_t[:, mc, :], outt[:, mc, :])

    nc.compile()
    return nc

def _rope_tables():
    pos = np.arange(S, dtype=np.float32)
    inv = 1.0 / (THETA ** (np.arange(0, Dh, 2, dtype=np.float32) / Dh))
    ang = pos[:, None] * inv[None, :]
    emb = np.concatenate([ang, ang], axis=-1)          # [S, Dh]
    cosT = np.cos(emb).T.astype(np.float32).copy()     # [Dh, S]
    ssinT = np.sin(emb).T.astype(np.float32).copy()
    ssinT[:64] = -ssinT[:64]
    return cosT, ssinT


def _tri_masks():
    # [128, 4, 512] for the diagonal 512-q-chunk, k-tile offset i in chunk:
    # col j: 0 if j < 128i; causal tri inside diag block; 1 past it.
    m = np.zeros((128, 4, 512), np.float32)
    for i in range(4):
        j = np.arange(512)[None, :]
        p = np.arange(128)[:, None]
        m[:, i, :] = ((j - 128 * i) >= p).astype(np.float32)
        m[:, i, : 128 * i] = 0.0
        m[:, i, 128 * (i + 1):] = 1.0
    return m.reshape(128, 4 * 512)


def kernel(**inputs):
    bf = ml_dtypes.bfloat16
    hs = np.ascontiguousarray(np.asarray(inputs["hidden_states"],
                                         np.float32)[0])
    ln1 = np.asarray(inputs["ln1_w"], np.float32)
    ln2 = np.asarray(inputs["ln2_w"], np.float32)
    Wq = np.asarray(inputs["Wq"], np.float32) * ln1[:, None]
    Wk = np.asarray(inputs["Wk"], np.float32) * ln1[:, None]
    Wv = np.asarray(inputs["Wv"], np.float32) * ln1[:, None]
    Wo = np.asarray(inputs["Wo"], np.float32)
    wg = np.asarray(inputs["w_gate"], np.float32) * ln2[:, None]
    wu = np.asarray(inputs["w_up"], np.float32) * ln2[:, None]
    wd = np.asarray(inputs["w_down"], np.float32)
    raw = np.asarray(inputs["router_attn_w"], np.float32)
    rab = np.asarray(inputs["router_attn_b"], np.float32)
    rmw = np.asarray(inputs["router_mlp_w"], np.float32)
    rmb = np.asarray(inputs["router_mlp_b"], np.float32)

    hsT = np.ascontiguousarray(hs.T)                   # [D, S]
    cosT, ssinT = _rope_tables()
    sc = np.float32(1.0 / np.sqrt(Dh))
    rd_a = (raw[:, 1] - raw[:, 0]).astype(np.float32)
    rd_m = (rmw[:, 1] - rmw[:, 0]).astype(np.float32)
    rd = np.concatenate([rd_a.reshape(NDT, 128).T, rd_m.reshape(NDT, 128).T],
                        axis=1).astype(np.float32)     # [128, 32]
    thr = np.array([[-(rab[1] - rab[0]), -(rmb[1] - rmb[0])]], np.float32)
    tri = np.ascontiguousarray(_tri_masks().astype(bf))
    qcos = np.ascontiguousarray(cosT.astype(bf))
    qsin = np.ascontiguousarray(ssinT.astype(bf))
    kcos = np.ascontiguousarray((cosT * sc).astype(bf))
    ksin = np.ascontiguousarray((ssinT * sc).astype(bf))

    if "nc" not in _CACHE:
        _CACHE["nc"] = _build_program()
    nc = _CACHE["nc"]

    in_maps = []
    for c in range(NC):
        dsl = slice(c * DCC, (c + 1) * DCC)
        fsl = slice(c * FPC, (c + 1) * FPC)
        in_maps.append({
            "hsT": hsT,
            "hres": np.ascontiguousarray(hsT[dsl]),
            "wq": np.ascontiguousarray(Wq[:, dsl].astype(bf)),
            "wk": np.ascontiguousarray(Wk[:, dsl].astype(bf)),
            "wv": np.ascontiguousarray(Wv[:, dsl].astype(bf)),
            "wo": np.ascontiguousarray(Wo[:, dsl].astype(bf)),
            "wg": np.ascontiguousarray(wg[:, fsl].astype(bf)),
            "wu": np.ascontiguousarray(wu[:, fsl].astype(bf)),
            "wd": np.ascontiguousarray(wd[fsl].astype(bf)),
            "qcos": qcos, "qsin": qsin, "kcos": kcos, "ksin": ksin,
            "tri": tri, "rd": rd, "thr": thr,
        })
    _CACHE["in_maps"] = in_maps
    res = run_bass_kernel_spmd(nc, in_maps, core_ids=list(range(NC)))
    _CACHE["res"] = res
    outT = np.concatenate([res.results[c]["out"] for c in range(NC)], axis=0)
    return np.ascontiguousarray(outT.T)[None]


if __name__ == "__main__":
    import reference
    inputs = reference.setup_inputs()
    out = kernel(**inputs)
    print(out.shape, out.dtype)



# revision 9
# speedup vs baseline: 2.0651x; 2.0651x over previous
"""Trainium2 Bass kernel v2 for nn_LlamaMoDDecoderLayer (MoD decoder layer).

Key design (8 cores, tensor-parallel, feature-major d-major layouts):
  - Routing (argmax masks) computed on HOST in fp64 (exact); device gets
    masks + compaction index tables as inputs. Compile is runtime-informed:
    compact capacity = ceil128(kept counts).
  - hsT shipped bf16; single stream. RMSNorm1 r1 folded into rope tables
    (q/k) and applied to v via per-token-column scale (r1col); QKV matmuls
    consume the raw bf16 hsT.
  - Attention dense (heads 2/core): transposed-scores causal softmax, exp
    via ACT, denominator via ones-matmul, reciprocal broadcast via
    ones-matmul into PSUM.
  - hs2 = hres + ma*(ctx@Wo) per-core rows (f32).  MLP COMPACTION: hs2
    transposed to token-major (PE transposes), kept-token columns gathered
    by dma_gather (SBUF transpose mode) -> compact [DCC, S2] -> AllGather
    -> norm2 -> gate/up/down on S2 tokens -> ReduceScatter [D, S2] ->
    per-core rs [DCC, S2] -> transpose + dma_gather expand back to [DCC, S]
    -> final residual with mm mask.
"""

import os

import numpy as np
import ml_dtypes

import concourse.bass as bass
import concourse.bacc as bacc
import concourse.mybir as mybir
import concourse.tile as tile
from concourse.alu_op_type import AluOpType
from concourse.bass_utils import run_bass_kernel_spmd

F32 = mybir.dt.float32
BF16 = mybir.dt.bfloat16
I16 = mybir.dt.int16
AF = mybir.ActivationFunctionType

S, D, H, Dh, F = 2048, 2048, 16, 128, 8192
NC = 8
HPC = H // NC            # heads per core (2)
DCC = D // NC            # output cols per core (256)
FPC = F // NC            # mlp hidden per core (1024)
NDT = D // 128           # 16 d-tiles
NFT = FPC // 128         # 8 local f-tiles
NSC = S // 512           # 4 s-chunks of 512
EPS = 1e-5
THETA = 10000.0

_CACHE = {}


def _chunks(total, step=512):
    """[(off, size), ...] covering total with `step`-sized chunks."""
    out = []
    off = 0
    while off < total:
        out.append((off, min(step, total - off)))
        off += step
    return out


def _build_program(s2, phases=9):
    """s2: compact token capacity for the MLP (multiple of 128)."""
    ns2t = s2 // 128
    nc = bacc.Bacc("TRN2", target_bir_lowering=False, debug=False,
                   num_devices=NC)
    rg = [list(range(NC))]

    d_xb = nc.dram_tensor("xb", [D, S], BF16, kind="ExternalInput")
    d_hresb = nc.dram_tensor("hresb", [DCC, S], BF16, kind="ExternalInput")
    d_wq = nc.dram_tensor("wq", [D, DCC], BF16, kind="ExternalInput")
    d_wk = nc.dram_tensor("wk", [D, DCC], BF16, kind="ExternalInput")
    d_wv = nc.dram_tensor("wv", [D, DCC], BF16, kind="ExternalInput")
    d_wo = nc.dram_tensor("wo", [D, DCC], BF16, kind="ExternalInput")
    d_wg = nc.dram_tensor("wg", [D, FPC], BF16, kind="ExternalInput")
    d_wu = nc.dram_tensor("wu", [D, FPC], BF16, kind="ExternalInput")
    d_wd = nc.dram_tensor("wd", [FPC, D], BF16, kind="ExternalInput")
    d_qcos = nc.dram_tensor("qcos", [Dh, S], BF16, kind="ExternalInput")
    d_qsin = nc.dram_tensor("qsin", [Dh, S], BF16, kind="ExternalInput")
    d_kcos = nc.dram_tensor("kcos", [Dh, S], BF16, kind="ExternalInput")
    d_ksin = nc.dram_tensor("ksin", [Dh, S], BF16, kind="ExternalInput")
    d_tri = nc.dram_tensor("tri", [128, 4 * 512], BF16, kind="ExternalInput")
    d_ma = nc.dram_tensor("ma", [128, S], BF16, kind="ExternalInput")
    d_ident = nc.dram_tensor("ident", [128, 128], BF16, kind="ExternalInput")
    d_gidxb = nc.dram_tensor("gidxb", [128, s2], F32, kind="ExternalInput")
    d_bidxb = nc.dram_tensor("bidxb", [128, S], F32, kind="ExternalInput")
    d_rowpos = nc.dram_tensor("rowpos", [128, NDT], F32,
                              kind="ExternalInput")
    d_out = nc.dram_tensor("out", [DCC, S], F32, kind="ExternalOutput")

    cc1_in = nc.dram_tensor("cc1_in", [DCC, S], BF16)
    cc1_out = nc.dram_tensor("cc1_out", [D, S], BF16, addr_space="Shared")
    cc2_in = nc.dram_tensor("cc2_in", [DCC, s2], BF16)
    cc2_out = nc.dram_tensor("cc2_out", [D, s2], BF16, addr_space="Shared")
    cc3_in = nc.dram_tensor("cc3_in", [D, s2], BF16)
    cc3_out = nc.dram_tensor("cc3_out", [DCC, s2], BF16)
    tc1_in = nc.dram_tensor("tc1_in", [16, 32], BF16)
    tc1_out = nc.dram_tensor("tc1_out", [128, 32], BF16, addr_space="Shared")
    tc2_in = nc.dram_tensor("tc2_in", [16, 32], BF16)
    tc2_out = nc.dram_tensor("tc2_out", [128, 32], BF16, addr_space="Shared")
    tc3_in = nc.dram_tensor("tc3_in", [128, 32], BF16)
    tc3_out = nc.dram_tensor("tc3_out", [16, 32], BF16)
    d_r1s = nc.dram_tensor("r1s", [1, S], F32)  # scratch for r1 transpose

    xb_t = d_xb.ap().rearrange("(a p) s -> p a s", p=128)
    hresb_t = d_hresb.ap().rearrange("(a p) s -> p a s", p=128)
    wq_t = d_wq.ap().rearrange("(a p) m -> p a m", p=128)
    wk_t = d_wk.ap().rearrange("(a p) m -> p a m", p=128)
    wv_t = d_wv.ap().rearrange("(a p) m -> p a m", p=128)
    wo_t = d_wo.ap().rearrange("(a p) m -> p a m", p=128)
    wg_t = d_wg.ap().rearrange("(a p) m -> p a m", p=128)
    wu_t = d_wu.ap().rearrange("(a p) m -> p a m", p=128)
    wd_t = d_wd.ap().rearrange("(a p) m -> p a m", p=128)
    cc1i_t = cc1_in.ap().rearrange("(a p) s -> p a s", p=128)
    cc2i_t = cc2_in.ap().rearrange("(a p) s -> p a s", p=128)
    cc3i_t = cc3_in.ap().rearrange("(a p) s -> p a s", p=128)
    cc1o_t = cc1_out.ap().rearrange("(a p) s -> p a s", p=128)
    cc2o_t = cc2_out.ap().rearrange("(a p) s -> p a s", p=128)
    cc3o_t = cc3_out.ap().rearrange("(a p) s -> p a s", p=128)
    out_t = d_out.ap().rearrange("(a p) s -> p a s", p=128)

    with tile.TileContext(nc) as tc:
        with (
            tc.tile_pool(name="const", bufs=1) as cst,
            tc.tile_pool(name="masks", bufs=1) as mkp,
        ):
            ones_b = cst.tile([128, 1], BF16)
            nc.gpsimd.memset(ones_b[:], 1.0)
            ones_r = cst.tile([1, 128], F32)
            nc.gpsimd.memset(ones_r[:], 1.0)
            eps1 = cst.tile([1, 1], F32)
            nc.gpsimd.memset(eps1[:], EPS)
            ident = cst.tile([128, 128], BF16, name="ident")
            nc.sync.dma_start(ident[:], d_ident.ap())
            ma_b = mkp.tile([128, S], BF16, name="ma_b")
            nc.sync.dma_start(ma_b[:], d_ma.ap())
            rowpos = cst.tile([128, NDT], F32, name="rowpos")
            nc.sync.dma_start(rowpos[:], d_rowpos.ap())
            hresb = mkp.tile([128, 2, S], BF16, name="hresb")
            nc.sync.dma_start(hresb[:], hresb_t)

            with (
                tc.tile_pool(name="attnconst", bufs=1) as acst,
                tc.tile_pool(name="xbp", bufs=1) as xbp,
            ):
                qcos = acst.tile([128, S], BF16, name="qcos")
                qsin = acst.tile([128, S], BF16, name="qsin")
                kcos = acst.tile([128, S], BF16, name="kcos")
                ksin = acst.tile([128, S], BF16, name="ksin")
                nc.sync.dma_start(qcos[:], d_qcos.ap())
                nc.sync.dma_start(qsin[:], d_qsin.ap())
                nc.sync.dma_start(kcos[:], d_kcos.ap())
                nc.sync.dma_start(ksin[:], d_ksin.ap())
                tri = acst.tile([128, 4, 512], BF16, name="tri")
                nc.sync.dma_start(
                    tri[:], d_tri.ap().rearrange("p (a m) -> p a m", m=512))
                xb = xbp.tile([128, NDT, S], BF16, name="xb")
                r1col = acst.tile([128, NDT], F32, name="r1col")

                # ---- phase 1: stream xb; sumsq -> r1; fold into tables ----
                with (
                    tc.tile_pool(name="ph1", bufs=1) as p1,
                    tc.tile_pool(name="ps1", bufs=1, space="PSUM") as ps1,
                ):
                    r1row = p1.tile([1, S], F32, name="r1row")
                    sqps = [ps1.tile([1, 512], F32, tag=f"sq{sc}", bufs=1,
                                     name=f"sqps{sc}")
                            for sc in range(NSC)]
                    for a in range(NDT):
                        nc.sync.dma_start(xb[:, a, :], xb_t[:, a, :])
                        sqt = p1.tile([128, S], BF16, tag="sq", bufs=3)
                        nc.scalar.activation(sqt[:], xb[:, a, :], AF.Square)
                        for sc in range(NSC):
                            nc.tensor.matmul(sqps[sc][:], ones_b[:],
                                             sqt[:, bass.ts(sc, 512)],
                                             start=(a == 0),
                                             stop=(a == NDT - 1))
                    for sc in range(NSC):
                        nc.scalar.activation(r1row[:, bass.ts(sc, 512)],
                                             sqps[sc][:], AF.Sqrt,
                                             bias=eps1[:], scale=1.0 / D)
                        nc.vector.reciprocal(r1row[:, bass.ts(sc, 512)],
                                             r1row[:, bass.ts(sc, 512)])
                    # r1 broadcast [128,S] (f32, psum chunks) folded into
                    # the 4 rope tables in place
                    for sc in range(NSC):
                        bcp = ps1.tile([128, 512], F32, tag="bc", bufs=2)
                        nc.tensor.matmul(bcp[:], ones_r[:],
                                         r1row[:, bass.ts(sc, 512)])
                        for tb in (qcos, qsin, kcos, ksin):
                            nc.vector.tensor_tensor(
                                tb[:, bass.ts(sc, 512)],
                                tb[:, bass.ts(sc, 512)], bcp[:],
                                op=AluOpType.mult)
                    # r1col [128, NDT]: token-major r1 via DRAM roundtrip
                    nc.sync.dma_start(d_r1s.ap(), r1row[:])
                    nc.sync.dma_start(
                        r1col[:], d_r1s.ap().rearrange("o (a p) -> p (o a)",
                                                       p=128))

                # ---- phase 2: QKV + rope ----
                with tc.tile_pool(name="qkv", bufs=1) as qkp:
                    if phases >= 2:
                        ps2cm = tc.tile_pool(name="ps2", bufs=1,
                                             space="PSUM")
                        ps2 = ps2cm.__enter__()
                        wqkv = qkp.tile([128, 3, NDT, DCC], BF16, name="wqkv")
                        nc.sync.dma_start(wqkv[:, 0], wq_t)
                        nc.sync.dma_start(wqkv[:, 1], wk_t)
                        nc.sync.dma_start(wqkv[:, 2], wv_t)
                        q_sb = qkp.tile([128, HPC, S], BF16, name="q_sb")
                        k_sb = qkp.tile([128, HPC, S], BF16, name="k_sb")
                        qs_sb = qkp.tile([128, HPC, S], BF16, name="qs_sb")
                        ks_sb = qkp.tile([128, HPC, S], BF16, name="ks_sb")
                        for wi, t_sb in ((0, q_sb), (1, k_sb)):
                            for mc in range(HPC):
                                for sc in range(NSC):
                                    ps = ps2.tile([128, 512], F32, tag="mm",
                                                  bufs=4)
                                    for a in range(NDT):
                                        nc.tensor.matmul(
                                            ps[:],
                                            wqkv[:, wi, a, bass.ts(mc, 128)],
                                            xb[:, a, bass.ts(sc, 512)],
                                            start=(a == 0),
                                            stop=(a == NDT - 1))
                                    nc.scalar.copy(
                                        t_sb[:, mc, bass.ts(sc, 512)],
                                        ps[:])
                        for src, dst in ((q_sb, qs_sb), (k_sb, ks_sb)):
                            for mc in range(HPC):
                                nc.sync.dma_start(dst[0:64, mc, :],
                                                  src[64:128, mc, :])
                                nc.sync.dma_start(dst[64:128, mc, :],
                                                  src[0:64, mc, :])
                        qr = qkp.tile([128, HPC, S], BF16, name="qr")
                        kr = qkp.tile([128, HPC, S], BF16, name="kr")
                        for mc in range(HPC):
                            tq = qkp.tile([128, S], BF16, tag="rt", bufs=2)
                            nc.vector.tensor_tensor(tq[:], qs_sb[:, mc, :],
                                                    qsin[:],
                                                    op=AluOpType.mult)
                            nc.vector.tensor_tensor(qr[:, mc, :],
                                                    q_sb[:, mc, :], qcos[:],
                                                    op=AluOpType.mult)
                            nc.vector.tensor_tensor(qr[:, mc, :],
                                                    qr[:, mc, :], tq[:],
                                                    op=AluOpType.add)
                            tk = qkp.tile([128, S], BF16, tag="rt", bufs=2)
                            nc.vector.tensor_tensor(tk[:], ks_sb[:, mc, :],
                                                    ksin[:],
                                                    op=AluOpType.mult)
                            nc.vector.tensor_tensor(kr[:, mc, :],
                                                    k_sb[:, mc, :], kcos[:],
                                                    op=AluOpType.mult)
                            nc.vector.tensor_tensor(kr[:, mc, :],
                                                    kr[:, mc, :], tk[:],
                                                    op=AluOpType.add)
                        v_sb = qkp.tile([128, NDT, DCC], BF16, name="v_sb")
                        for mc in range(NDT):
                            ps = ps2.tile([128, DCC], F32, tag="mmv",
                                          bufs=3)
                            for a in range(NDT):
                                nc.tensor.matmul(
                                    ps[:], xb[:, a, bass.ts(mc, 128)],
                                    wqkv[:, 2, a, :],
                                    start=(a == 0), stop=(a == NDT - 1))
                            nc.vector.tensor_scalar(
                                v_sb[:, mc, :], ps[:],
                                r1col[:, mc:mc + 1], None,
                                op0=AluOpType.mult)
                        ps2cm.__exit__(None, None, None)

                    # ---- phase 3: attention ----
                    if phases >= 3:
                        ps3cm = tc.tile_pool(name="ps3", bufs=1,
                                             space="PSUM")
                        ps3 = ps3cm.__enter__()
                        ctxT = qkp.tile([128, HPC, S], BF16, name="ctxT")
                        if os.environ.get("DUMMY_ATTN"):
                            for h in range(HPC):
                                nc.scalar.copy(ctxT[:, h, :], qr[:, h, :])
                        else:
                            _full_attn = True
                        for h in range(HPC if not os.environ.get("DUMMY_ATTN")
                                       else 0):
                            for qc in range(NSC):
                                nkt = 4 * (qc + 1)
                                cps = ps3.tile([128, 512], F32, tag="ctx",
                                               bufs=2)
                                dps = ps3.tile([1, 512], F32, tag="den",
                                               bufs=2)
                                for kt in range(nkt):
                                    sps = ps3.tile([128, 512], F32,
                                                   tag="st", bufs=3)
                                    nc.tensor.matmul(
                                        sps[:], kr[:, h, bass.ts(kt, 128)],
                                        qr[:, h, bass.ts(qc, 512)])
                                    est = qkp.tile([128, 512], BF16,
                                                   tag="est", bufs=4)
                                    nc.scalar.activation(est[:], sps[:],
                                                         AF.Exp)
                                    if kt // 4 == qc:
                                        nc.vector.tensor_tensor(
                                            est[:], est[:], tri[:, kt % 4, :],
                                            op=AluOpType.mult)
                                    nc.tensor.matmul(
                                        cps[:], v_sb[:, kt, bass.ts(h, 128)],
                                        est[:], start=(kt == 0),
                                        stop=(kt == nkt - 1))
                                    nc.tensor.matmul(
                                        dps[:], ones_b[:], est[:],
                                        start=(kt == 0),
                                        stop=(kt == nkt - 1))
                                rrow = qkp.tile([1, 512], F32, tag="rr",
                                                bufs=2)
                                nc.vector.reciprocal(rrow[:], dps[:])
                                rbs = qkp.tile([128, 512], F32, tag="rbs",
                                               bufs=2)
                                nc.gpsimd.partition_broadcast(rbs[:],
                                                              rrow[:])
                                nc.vector.tensor_tensor(
                                    ctxT[:, h, bass.ts(qc, 512)], cps[:],
                                    rbs[:], op=AluOpType.mult)
                        for mc in range(HPC):
                            nc.sync.dma_start(cc1i_t[:, mc, :],
                                              ctxT[:, mc, :])
                        ps3cm.__exit__(None, None, None)

            # ---- phase 4: AG ctx + Wo + hs2 + transpose + gather ----
            if phases >= 4 and not os.environ.get("NO_CC"):
                if os.environ.get("TINY_CC"):
                    nc.gpsimd.collective_compute(
                        "AllGather", AluOpType.bypass, replica_groups=rg,
                        ins=[tc1_in.ap()], outs=[tc1_out.ap()])
                else:
                    nc.gpsimd.collective_compute(
                        "AllGather", AluOpType.bypass, replica_groups=rg,
                        ins=[cc1_in.ap()], outs=[cc1_out.ap()])
            with tc.tile_pool(name="p4", bufs=1) as p4:
                if phases >= 4:
                    hs2f = p4.tile([128, 2, S], F32, name="hs2f")
                    hs2tok = p4.tile([128, NDT, DCC], BF16, name="hs2tok")
                    ps4acm = tc.tile_pool(name="ps4a", bufs=1,
                                          space="PSUM")
                    ps4a = ps4acm.__enter__()
                    wopcm = tc.tile_pool(name="wo_ph", bufs=1)
                    wop = wopcm.__enter__()
                    gidxb = wop.tile([128, s2], F32, name="gidxb")
                    nc.sync.dma_start(gidxb[:], d_gidxb.ap())
                    ctxg = wop.tile([128, NDT, S], BF16, name="ctxg")
                    for a in range(NDT):
                        nc.sync.dma_start(ctxg[:, a, :], cc1o_t[:, a, :])
                    wo = wop.tile([128, NDT, DCC], BF16, name="wo")
                    nc.sync.dma_start(wo[:], wo_t)
                    wops = [ps4a.tile([128, 512], F32, tag=f"wo{mc}{sc}",
                                      bufs=1, name=f"wops{mc}{sc}")
                            for mc in range(HPC) for sc in range(NSC)]
                    for a in range(NDT):
                        for mc in range(HPC):
                            for sc in range(NSC):
                                nc.tensor.matmul(
                                    wops[mc * NSC + sc][:],
                                    wo[:, a, bass.ts(mc, 128)],
                                    ctxg[:, a, bass.ts(sc, 512)],
                                    start=(a == 0), stop=(a == NDT - 1))
                    for mc in range(HPC):
                        for sc in range(NSC):
                            ps = wops[mc * NSC + sc]
                            t = wop.tile([128, 512], F32, tag="wot",
                                         bufs=2)
                            nc.vector.tensor_tensor(
                                t[:], ps[:], ma_b[:, bass.ts(sc, 512)],
                                op=AluOpType.mult)
                            nc.vector.tensor_tensor(
                                hs2f[:, mc, bass.ts(sc, 512)], t[:],
                                hresb[:, mc, bass.ts(sc, 512)],
                                op=AluOpType.add)
                    ps4acm.__exit__(None, None, None)
                    ps4cm = tc.tile_pool(name="ps4", bufs=1, space="PSUM")
                    ps4 = ps4cm.__enter__()
                    # hs2 token-major (bf16) via PE transposes
                    hs2b = wop.tile([128, 2, S], BF16, name="hs2b")
                    for mc in range(HPC):
                        nc.scalar.copy(hs2b[:, mc, :], hs2f[:, mc, :])
                    if not os.environ.get("DUMMY_GATHER"):
                        for dc in range(HPC):
                            for tcix in range(NDT):
                                tps = ps4.tile([128, 128], BF16, tag="tp",
                                               bufs=2)
                                nc.tensor.transpose(
                                    tps[:], hs2b[:, dc, bass.ts(tcix, 128)],
                                    ident[:])
                                nc.scalar.copy(
                                    hs2tok[:, tcix, bass.ts(dc, 128)],
                                    tps[:])
                    # compact via selection matmuls:
                    # xc[d, j] = sum_t hs2tok[t, d] * (gidxb[j] == t)
                    xc = wop.tile([128, 2, s2], BF16, name="xc")
                    if os.environ.get("DUMMY_GATHER"):
                        for mc in range(HPC):
                            nc.scalar.copy(xc[:, mc, :],
                                           hs2b[:, mc, 0:s2])
                    for off, sz in ([] if os.environ.get("DUMMY_GATHER")
                                    else _chunks(s2)):
                        ps0 = ps4.tile([128, 512], F32, tag="gx0", bufs=1)
                        ps1 = ps4.tile([128, 512], F32, tag="gx1", bufs=1)
                        for tt in range(NDT):
                            sel = wop.tile([128, 512], BF16, tag="sel",
                                           bufs=3)
                            nc.vector.tensor_scalar(
                                sel[:, 0:sz], gidxb[:, off:off + sz],
                                rowpos[:, tt:tt + 1], None,
                                op0=AluOpType.is_equal)
                            nc.tensor.matmul(
                                ps0[:, 0:sz], hs2tok[:, tt, 0:128],
                                sel[:, 0:sz], start=(tt == 0),
                                stop=(tt == NDT - 1))
                            nc.tensor.matmul(
                                ps1[:, 0:sz], hs2tok[:, tt, 128:256],
                                sel[:, 0:sz], start=(tt == 0),
                                stop=(tt == NDT - 1))
                        nc.scalar.copy(xc[:, 0, off:off + sz], ps0[:, 0:sz])
                        nc.scalar.copy(xc[:, 1, off:off + sz], ps1[:, 0:sz])
                    for mc in range(HPC):
                        nc.sync.dma_start(cc2i_t[:, mc, :], xc[:, mc, :])
                    wopcm.__exit__(None, None, None)
                    ps4cm.__exit__(None, None, None)
                if phases >= 5:
                    if not os.environ.get("NO_CC"):
                        if os.environ.get("TINY_CC"):
                            nc.gpsimd.collective_compute(
                                "AllGather", AluOpType.bypass,
                                replica_groups=rg,
                                ins=[tc2_in.ap()], outs=[tc2_out.ap()])
                        else:
                            nc.gpsimd.collective_compute(
                                "AllGather", AluOpType.bypass,
                                replica_groups=rg,
                                ins=[cc2_in.ap()], outs=[cc2_out.ap()])

                    # ---- phase 5: norm2 + MLP on compact tokens ----
                    with (
                        tc.tile_pool(name="mlp", bufs=1) as mlp,
                        tc.tile_pool(name="ps5", bufs=1,
                                     space="PSUM") as ps5,
                    ):
                        hs2g = mlp.tile([128, NDT, s2], BF16, name="hs2g")
                        for a in range(NDT):
                            nc.sync.dma_start(hs2g[:, a, :], cc2o_t[:, a, :])
                        with tc.tile_pool(name="r2p", bufs=1) as r2p:
                            r2row = r2p.tile([1, s2], F32, name="r2row")
                            r2b = r2p.tile([128, s2], F32, name="r2b")
                            for sc, (off, sz) in enumerate(_chunks(s2)):
                                ssp = ps5.tile([1, 512], F32, tag="ss",
                                               bufs=2)
                                for a in range(NDT):
                                    sqt = r2p.tile([128, 512], BF16,
                                                   tag="sq2", bufs=3)
                                    nc.scalar.activation(
                                        sqt[:, 0:sz],
                                        hs2g[:, a, off:off + sz], AF.Square)
                                    nc.tensor.matmul(ssp[:, 0:sz], ones_b[:],
                                                     sqt[:, 0:sz],
                                                     start=(a == 0),
                                                     stop=(a == NDT - 1))
                                nc.scalar.activation(r2row[:, off:off + sz],
                                                     ssp[:, 0:sz], AF.Sqrt,
                                                     bias=eps1[:],
                                                     scale=1.0 / D)
                                nc.vector.reciprocal(r2row[:, off:off + sz],
                                                     r2row[:, off:off + sz])
                                bcp = ps5.tile([128, 512], F32, tag="bc2",
                                               bufs=1)
                                nc.tensor.matmul(bcp[:, 0:sz], ones_r[:],
                                                 r2row[:, off:off + sz])
                                nc.vector.tensor_copy(r2b[:, off:off + sz],
                                                      bcp[:, 0:sz])
                            for a in range(NDT):
                                nc.vector.tensor_tensor(
                                    hs2g[:, a, :], hs2g[:, a, :], r2b[:],
                                    op=AluOpType.mult)
                        xn2 = hs2g
                        hT = mlp.tile([128, NFT, s2], BF16, name="hT")
                        if os.environ.get("DUMMY_MLP"):
                            for fc in range(NFT):
                                nc.scalar.copy(hT[:, fc, :], xn2[:, fc, :])
                            stgd = mlp.tile([128, s2], BF16, name="stgd")
                            for mc in range(NDT):
                                nc.scalar.copy(stgd[:], hT[:, mc % NFT, :])
                                nc.sync.dma_start(cc3i_t[:, mc, :], stgd[:])
                        _skip_mlp = bool(os.environ.get("DUMMY_MLP"))
                        with tc.tile_pool(name="wstream", bufs=3) as wsp:
                          if not _skip_mlp:
                            for fc in range(NFT):
                                wgc = wsp.tile([128, NDT, 128], BF16,
                                               tag="wgc")
                                nc.sync.dma_start(
                                    wgc[:], wg_t[:, :, bass.ts(fc, 128)])
                                sg = wsp.tile([128, s2], BF16, tag="sg",
                                              bufs=2)
                                for off, sz in _chunks(s2):
                                    ps = ps5.tile([128, 512], F32, tag="mm",
                                                  bufs=4)
                                    for a in range(NDT):
                                        nc.tensor.matmul(
                                            ps[:, 0:sz], wgc[:, a, :],
                                            xn2[:, a, off:off + sz],
                                            start=(a == 0),
                                            stop=(a == NDT - 1))
                                    sgs = wsp.tile([128, 512], BF16,
                                                   tag="sgs", bufs=2)
                                    nc.scalar.activation(sgs[:, 0:sz],
                                                         ps[:, 0:sz],
                                                         AF.Sigmoid)
                                    nc.vector.tensor_tensor(
                                        sg[:, off:off + sz], ps[:, 0:sz],
                                        sgs[:, 0:sz], op=AluOpType.mult)
                                wuc = wsp.tile([128, NDT, 128], BF16,
                                               tag="wuc")
                                nc.sync.dma_start(
                                    wuc[:], wu_t[:, :, bass.ts(fc, 128)])
                                for off, sz in _chunks(s2):
                                    ps = ps5.tile([128, 512], F32, tag="mm",
                                                  bufs=4)
                                    for a in range(NDT):
                                        nc.tensor.matmul(
                                            ps[:, 0:sz], wuc[:, a, :],
                                            xn2[:, a, off:off + sz],
                                            start=(a == 0),
                                            stop=(a == NDT - 1))
                                    nc.vector.tensor_tensor(
                                        hT[:, fc, off:off + sz], ps[:, 0:sz],
                                        sg[:, off:off + sz],
                                        op=AluOpType.mult)
                            for mc in range(NDT):
                                wdc = wsp.tile([128, NFT, 128], BF16,
                                               tag="wdc")
                                nc.sync.dma_start(
                                    wdc[:], wd_t[:, :, bass.ts(mc, 128)])
                                stg = wsp.tile([128, s2], BF16,
                                               tag="stg", bufs=3)
                                for off, sz in _chunks(s2):
                                    ps = ps5.tile([128, 512], F32, tag="mm",
                                                  bufs=4)
                                    for a in range(NFT):
                                        nc.tensor.matmul(
                                            ps[:, 0:sz], wdc[:, a, :],
                                            hT[:, a, off:off + sz],
                                            start=(a == 0),
                                            stop=(a == NFT - 1))
                                    nc.scalar.copy(stg[:, off:off + sz],
                                                   ps[:, 0:sz])
                                nc.sync.dma_start(cc3i_t[:, mc, :], stg[:])
                if phases >= 6:
                    if not os.environ.get("NO_CC"):
                        if os.environ.get("TINY_CC"):
                            nc.gpsimd.collective_compute(
                                "ReduceScatter", AluOpType.add,
                                replica_groups=rg,
                                ins=[tc3_in.ap()], outs=[tc3_out.ap()])
                        else:
                            nc.gpsimd.collective_compute(
                                "ReduceScatter", AluOpType.add,
                                replica_groups=rg,
                                ins=[cc3_in.ap()], outs=[cc3_out.ap()])

                    # ---- phase 6: expand + final residual ----
                    with (
                        tc.tile_pool(name="fin", bufs=1) as fin,
                        tc.tile_pool(name="ps6", bufs=1,
                                     space="PSUM") as ps6,
                    ):
                        bidxb = fin.tile([128, S], F32, name="bidxb")
                        nc.sync.dma_start(bidxb[:], d_bidxb.ap())
                        rs = fin.tile([128, 2, s2], BF16, name="rs")
                        for mc in range(HPC):
                            nc.sync.dma_start(rs[:, mc, :], cc3o_t[:, mc, :])
                        rstok = fin.tile([128, ns2t, DCC], BF16,
                                         name="rstok")
                        if not os.environ.get("DUMMY_GATHER"):
                         for dc in range(HPC):
                            for tcix in range(ns2t):
                                tps = ps6.tile([128, 128], BF16, tag="tp2",
                                               bufs=3)
                                nc.tensor.transpose(
                                    tps[:], rs[:, dc, bass.ts(tcix, 128)],
                                    ident[:])
                                nc.scalar.copy(
                                    rstok[:, tcix, bass.ts(dc, 128)], tps[:])
                        # expand via selection matmuls fused with residual:
                        # out[d, s] = hs2f[d, s]
                        #             + sum_ct rstok[ct, d]*(bidxb[s] == ct)
                        outt = fin.tile([128, 2, S], F32, name="outt")
                        if os.environ.get("DUMMY_GATHER"):
                            for mc in range(HPC):
                                t9 = fin.tile([128, S], F32, tag="t9",
                                              bufs=2)
                                nc.vector.tensor_tensor(
                                    t9[:, 0:s2], rs[:, mc, :],
                                    hs2f[:, mc, 0:s2], op=AluOpType.add)
                                nc.vector.tensor_copy(outt[:, mc, :],
                                                      hs2f[:, mc, :])
                        for off, sz in ([] if os.environ.get("DUMMY_GATHER")
                                        else _chunks(S)):
                            ps0 = ps6.tile([128, 512], F32, tag="ex0",
                                           bufs=2)
                            ps1 = ps6.tile([128, 512], F32, tag="ex1",
                                           bufs=2)
                            for ct in range(ns2t):
                                sel = fin.tile([128, 512], BF16, tag="sel2",
                                               bufs=3)
                                nc.vector.tensor_scalar(
                                    sel[:, 0:sz], bidxb[:, off:off + sz],
                                    rowpos[:, ct:ct + 1], None,
                                    op0=AluOpType.is_equal)
                                nc.tensor.matmul(
                                    ps0[:, 0:sz], rstok[:, ct, 0:128],
                                    sel[:, 0:sz], start=(ct == 0),
                                    stop=(ct == ns2t - 1))
                                nc.tensor.matmul(
                                    ps1[:, 0:sz], rstok[:, ct, 128:256],
                                    sel[:, 0:sz], start=(ct == 0),
                                    stop=(ct == ns2t - 1))
                            nc.vector.tensor_tensor(
                                outt[:, 0, off:off + sz], ps0[:, 0:sz],
                                hs2f[:, 0, off:off + sz], op=AluOpType.add)
                            nc.vector.tensor_tensor(
                                outt[:, 1, off:off + sz], ps1[:, 0:sz],
                                hs2f[:, 1, off:off + sz], op=AluOpType.add)
                        for mc in range(HPC):
                            nc.sync.dma_start(out_t[:, mc, :], outt[:, mc, :])

    nc.compile()
    return nc


def _rope_tables():
    pos = np.arange(S, dtype=np.float32)
    inv = 1.0 / (THETA ** (np.arange(0, Dh, 2, dtype=np.float32) / Dh))
    ang = pos[:, None] * inv[None, :]
    emb = np.concatenate([ang, ang], axis=-1)          # [S, Dh]
    cosT = np.cos(emb).T.astype(np.float32).copy()     # [Dh, S]
    ssinT = np.sin(emb).T.astype(np.float32).copy()
    ssinT[:64] = -ssinT[:64]
    return cosT, ssinT


def _tri_masks():
    m = np.zeros((128, 4, 512), np.float32)
    for i in range(4):
        j = np.arange(512)[None, :]
        p = np.arange(128)[:, None]
        m[:, i, :] = ((j - 128 * i) >= p).astype(np.float32)
        m[:, i, : 128 * i] = 0.0
        m[:, i, 128 * (i + 1):] = 1.0
    return m.reshape(128, 4 * 512)


def _wrap_idx(vals, n):
    """Index layout for dma_gather: [128, n//16] int16, idx j at partition
    j%16 (replicated across the 8 16-partition groups), free pos j//16."""
    t = np.zeros((128, n // 16), np.int16)
    v = np.asarray(vals, np.int16)
    for grp in range(8):
        t[grp * 16:(grp + 1) * 16, :] = v.reshape(n // 16, 16).T
    return t


def kernel(**inputs):
    bf = ml_dtypes.bfloat16
    hs = np.ascontiguousarray(np.asarray(inputs["hidden_states"],
                                         np.float32)[0])
    ln1 = np.asarray(inputs["ln1_w"], np.float32)
    ln2 = np.asarray(inputs["ln2_w"], np.float32)
    Wq = np.asarray(inputs["Wq"], np.float32) * ln1[:, None]
    Wk = np.asarray(inputs["Wk"], np.float32) * ln1[:, None]
    Wv = np.asarray(inputs["Wv"], np.float32) * ln1[:, None]
    Wo = np.asarray(inputs["Wo"], np.float32)
    wg = np.asarray(inputs["w_gate"], np.float32) * ln2[:, None]
    wu = np.asarray(inputs["w_up"], np.float32) * ln2[:, None]
    wd = np.asarray(inputs["w_down"], np.float32)
    raw = np.asarray(inputs["router_attn_w"], np.float64)
    rab = np.asarray(inputs["router_attn_b"], np.float64)
    rmw = np.asarray(inputs["router_mlp_w"], np.float64)
    rmb = np.asarray(inputs["router_mlp_b"], np.float64)

    # host-side routing (fp64; exact vs fp32 reference since margins >> eps)
    la = hs.astype(np.float64) @ raw + rab
    lm = hs.astype(np.float64) @ rmw + rmb
    route_attn = la[:, 1] > la[:, 0]          # True -> attn output zeroed
    route_mlp = lm[:, 1] > lm[:, 0]           # True -> mlp skipped
    ma = (~route_attn).astype(np.float32)     # 1 -> keep attn out
    mm = (~route_mlp).astype(np.float32)      # 1 -> apply mlp
    kept = np.nonzero(mm)[0]
    cnt = len(kept)
    s2 = max(128, -(-cnt // 128) * 128)
    gidx_vals = np.zeros(s2, np.int64)
    gidx_vals[:cnt] = kept
    back = np.zeros(S, np.int64)
    back[kept] = np.arange(cnt)

    hsT = np.ascontiguousarray(hs.T)                   # [D, S]
    cosT, ssinT = _rope_tables()
    sc = np.float32(1.0 / np.sqrt(Dh))
    tri = np.ascontiguousarray(_tri_masks().astype(bf))
    qcos = np.ascontiguousarray(cosT.astype(bf))
    qsin = np.ascontiguousarray(ssinT.astype(bf))
    kcos = np.ascontiguousarray((cosT * sc).astype(bf))
    ksin = np.ascontiguousarray((ssinT * sc).astype(bf))
    ma_b = np.ascontiguousarray(
        np.broadcast_to(ma[None, :], (128, S)).astype(bf))
    ident = np.eye(128, dtype=np.float32).astype(bf)
    gidxv = np.full(s2, -1.0, np.float32)
    gidxv[:cnt] = kept.astype(np.float32)
    gidxb = np.ascontiguousarray(
        np.broadcast_to(gidxv[None, :], (128, s2)).astype(np.float32))
    bidxv = np.full(S, -1.0, np.float32)
    bidxv[kept] = np.arange(cnt, dtype=np.float32)
    bidxb = np.ascontiguousarray(
        np.broadcast_to(bidxv[None, :], (128, S)).astype(np.float32))
    rowpos = np.ascontiguousarray(
        (np.arange(128, dtype=np.float32)[:, None]
         + 128.0 * np.arange(NDT, dtype=np.float32)[None, :]))

    key = ("nc", s2)
    if key not in _CACHE:
        _CACHE["nc"] = _build_program(s2)
        _CACHE["key"] = key
    nc = _CACHE["nc"]

    xb_full = np.ascontiguousarray(hsT.astype(bf))
    in_maps = []
    for c in range(NC):
        dsl = slice(c * DCC, (c + 1) * DCC)
        fsl = slice(c * FPC, (c + 1) * FPC)
        in_maps.append({
            "xb": xb_full,
            "hresb": np.ascontiguousarray(xb_full[dsl]),
            "wq": np.ascontiguousarray(Wq[:, dsl].astype(bf)),
            "wk": np.ascontiguousarray(Wk[:, dsl].astype(bf)),
            "wv": np.ascontiguousarray(Wv[:, dsl].astype(bf)),
            "wo": np.ascontiguousarray(Wo[:, dsl].astype(bf)),
            "wg": np.ascontiguousarray(wg[:, fsl].astype(bf)),
            "wu": np.ascontiguousarray(wu[:, fsl].astype(bf)),
            "wd": np.ascontiguousarray(wd[fsl].astype(bf)),
            "qcos": qcos, "qsin": qsin, "kcos": kcos, "ksin": ksin,
            "tri": tri, "ma": ma_b, "ident": ident,
            "gidxb": gidxb, "bidxb": bidxb, "rowpos": rowpos,
        })
    _CACHE["in_maps"] = in_maps
    res = run_bass_kernel_spmd(nc, in_maps, core_ids=list(range(NC)))
    _CACHE["res"] = res
    outT = np.concatenate([res.results[c]["out"] for c in range(NC)], axis=0)
    return np.ascontiguousarray(outT.T)[None]


if __name__ == "__main__":
    import reference
    inputs = reference.setup_inputs()
    out = kernel(**inputs)
    print(out.shape, out.dtype)


# revision 10
# speedup vs baseline: 2.2794x; 1.1038x over previous
"""Trainium2 Bass kernel v2 for nn_LlamaMoDDecoderLayer (MoD decoder layer).

Key design (8 cores, tensor-parallel, feature-major d-major layouts):
  - Routing (argmax masks) computed on HOST in fp64 (exact); device gets
    masks + compaction index tables as inputs. Compile is runtime-informed:
    compact capacity = ceil128(kept counts).
  - hsT shipped bf16; single stream. RMSNorm1 r1 folded into rope tables
    (q/k) and applied to v via per-token-column scale (r1col); QKV matmuls
    consume the raw bf16 hsT.
  - Attention dense (heads 2/core): transposed-scores causal softmax, exp
    via ACT, denominator via ones-matmul, reciprocal broadcast via
    ones-matmul into PSUM.
  - hs2 = hres + ma*(ctx@Wo) per-core rows (f32).  MLP COMPACTION: hs2
    transposed to token-major (PE transposes), kept-token columns gathered
    by dma_gather (SBUF transpose mode) -> compact [DCC, S2] -> AllGather
    -> norm2 -> gate/up/down on S2 tokens -> ReduceScatter [D, S2] ->
    per-core rs [DCC, S2] -> transpose + dma_gather expand back to [DCC, S]
    -> final residual with mm mask.
"""

import os

import numpy as np
import ml_dtypes

import concourse.bass as bass
import concourse.bacc as bacc
import concourse.mybir as mybir
import concourse.tile as tile
from concourse.alu_op_type import AluOpType
from concourse.bass_utils import run_bass_kernel_spmd

F32 = mybir.dt.float32
BF16 = mybir.dt.bfloat16
I16 = mybir.dt.int16
AF = mybir.ActivationFunctionType

S, D, H, Dh, F = 2048, 2048, 16, 128, 8192
NC = 8
HPC = H // NC            # heads per core (2)
DCC = D // NC            # output cols per core (256)
FPC = F // NC            # mlp hidden per core (1024)
NDT = D // 128           # 16 d-tiles
NFT = FPC // 128         # 8 local f-tiles
NSC = S // 512           # 4 s-chunks of 512
EPS = 1e-5
THETA = 10000.0

_CACHE = {}


def _chunks(total, step=512):
    """[(off, size), ...] covering total with `step`-sized chunks."""
    out = []
    off = 0
    while off < total:
        out.append((off, min(step, total - off)))
        off += step
    return out


def _build_program(s2, phases=9):
    """s2: compact token capacity for the MLP (multiple of 128)."""
    ns2t = s2 // 128
    nc = bacc.Bacc("TRN2", target_bir_lowering=False, debug=False,
                   num_devices=NC)
    rg = [list(range(NC))]

    d_xb = nc.dram_tensor("xb", [D, S], BF16, kind="ExternalInput")
    d_hresb = nc.dram_tensor("hresb", [DCC, S], BF16, kind="ExternalInput")
    d_wq = nc.dram_tensor("wq", [D, DCC], BF16, kind="ExternalInput")
    d_wk = nc.dram_tensor("wk", [D, DCC], BF16, kind="ExternalInput")
    d_wv = nc.dram_tensor("wv", [D, DCC], BF16, kind="ExternalInput")
    d_wo = nc.dram_tensor("wo", [D, DCC], BF16, kind="ExternalInput")
    d_wg = nc.dram_tensor("wg", [D, FPC], BF16, kind="ExternalInput")
    d_wu = nc.dram_tensor("wu", [D, FPC], BF16, kind="ExternalInput")
    d_wd = nc.dram_tensor("wd", [FPC, D], BF16, kind="ExternalInput")
    d_qcos = nc.dram_tensor("qcos", [Dh, S], BF16, kind="ExternalInput")
    d_qsin = nc.dram_tensor("qsin", [Dh, S], BF16, kind="ExternalInput")
    d_kcos = nc.dram_tensor("kcos", [Dh, S], BF16, kind="ExternalInput")
    d_ksin = nc.dram_tensor("ksin", [Dh, S], BF16, kind="ExternalInput")
    d_tri = nc.dram_tensor("tri", [128, 4 * 512], BF16, kind="ExternalInput")
    d_ma = nc.dram_tensor("ma", [128, S], BF16, kind="ExternalInput")
    d_ident = nc.dram_tensor("ident", [128, 128], BF16, kind="ExternalInput")
    d_gidxb = nc.dram_tensor("gidxb", [128, s2], F32, kind="ExternalInput")
    d_bidxb = nc.dram_tensor("bidxb", [128, S], F32, kind="ExternalInput")
    d_rowpos = nc.dram_tensor("rowpos", [128, NDT], F32,
                              kind="ExternalInput")
    d_out = nc.dram_tensor("out", [DCC, S], F32, kind="ExternalOutput")

    cc1_in = nc.dram_tensor("cc1_in", [DCC, S], BF16)
    cc1_out = nc.dram_tensor("cc1_out", [D, S], BF16, addr_space="Shared")
    cc2_in = nc.dram_tensor("cc2_in", [DCC, s2], BF16)
    cc2_out = nc.dram_tensor("cc2_out", [D, s2], BF16, addr_space="Shared")
    cc3_in = nc.dram_tensor("cc3_in", [D, s2], BF16)
    cc3_out = nc.dram_tensor("cc3_out", [DCC, s2], BF16)
    tc1_in = nc.dram_tensor("tc1_in", [16, 32], BF16)
    tc1_out = nc.dram_tensor("tc1_out", [128, 32], BF16, addr_space="Shared")
    tc2_in = nc.dram_tensor("tc2_in", [16, 32], BF16)
    tc2_out = nc.dram_tensor("tc2_out", [128, 32], BF16, addr_space="Shared")
    tc3_in = nc.dram_tensor("tc3_in", [128, 32], BF16)
    tc3_out = nc.dram_tensor("tc3_out", [16, 32], BF16)
    d_r1s = nc.dram_tensor("r1s", [1, S], F32)  # scratch for r1 transpose

    xb_t = d_xb.ap().rearrange("(a p) s -> p a s", p=128)
    hresb_t = d_hresb.ap().rearrange("(a p) s -> p a s", p=128)
    wq_t = d_wq.ap().rearrange("(a p) m -> p a m", p=128)
    wk_t = d_wk.ap().rearrange("(a p) m -> p a m", p=128)
    wv_t = d_wv.ap().rearrange("(a p) m -> p a m", p=128)
    wo_t = d_wo.ap().rearrange("(a p) m -> p a m", p=128)
    wg_t = d_wg.ap().rearrange("(a p) m -> p a m", p=128)
    wu_t = d_wu.ap().rearrange("(a p) m -> p a m", p=128)
    wd_t = d_wd.ap().rearrange("(a p) m -> p a m", p=128)
    cc1i_t = cc1_in.ap().rearrange("(a p) s -> p a s", p=128)
    cc2i_t = cc2_in.ap().rearrange("(a p) s -> p a s", p=128)
    cc3i_t = cc3_in.ap().rearrange("(a p) s -> p a s", p=128)
    cc1o_t = cc1_out.ap().rearrange("(a p) s -> p a s", p=128)
    cc2o_t = cc2_out.ap().rearrange("(a p) s -> p a s", p=128)
    cc3o_t = cc3_out.ap().rearrange("(a p) s -> p a s", p=128)
    out_t = d_out.ap().rearrange("(a p) s -> p a s", p=128)

    with tile.TileContext(nc) as tc:
        with (
            tc.tile_pool(name="const", bufs=1) as cst,
            tc.tile_pool(name="masks", bufs=1) as mkp,
        ):
            ones_b = cst.tile([128, 1], BF16)
            nc.gpsimd.memset(ones_b[:], 1.0)
            ones_r = cst.tile([1, 128], F32)
            nc.gpsimd.memset(ones_r[:], 1.0)
            eps1 = cst.tile([1, 1], F32)
            nc.gpsimd.memset(eps1[:], EPS)
            ident = cst.tile([128, 128], BF16, name="ident")
            nc.sync.dma_start(ident[:], d_ident.ap())
            ma_b = mkp.tile([128, S], BF16, name="ma_b")
            nc.sync.dma_start(ma_b[:], d_ma.ap())
            rowpos = cst.tile([128, NDT], F32, name="rowpos")
            nc.sync.dma_start(rowpos[:], d_rowpos.ap())
            hresb = mkp.tile([128, 2, S], BF16, name="hresb")
            nc.sync.dma_start(hresb[:], hresb_t)

            with (
                tc.tile_pool(name="attnconst", bufs=1) as acst,
                tc.tile_pool(name="xbp", bufs=1) as xbp,
            ):
                qcos = acst.tile([128, S], BF16, name="qcos")
                qsin = acst.tile([128, S], BF16, name="qsin")
                kcos = acst.tile([128, S], BF16, name="kcos")
                ksin = acst.tile([128, S], BF16, name="ksin")
                nc.sync.dma_start(qcos[:], d_qcos.ap())
                nc.sync.dma_start(qsin[:], d_qsin.ap())
                nc.sync.dma_start(kcos[:], d_kcos.ap())
                nc.sync.dma_start(ksin[:], d_ksin.ap())
                tri = acst.tile([128, 4, 512], BF16, name="tri")
                nc.sync.dma_start(
                    tri[:], d_tri.ap().rearrange("p (a m) -> p a m", m=512))
                xb = xbp.tile([128, NDT, S], BF16, name="xb")
                r1col = acst.tile([128, NDT], F32, name="r1col")

                # ---- phase 1: stream xb; sumsq -> r1; fold into tables ----
                with (
                    tc.tile_pool(name="ph1", bufs=1) as p1,
                    tc.tile_pool(name="ps1", bufs=1, space="PSUM") as ps1,
                ):
                    r1row = p1.tile([1, S], F32, name="r1row")
                    sqps = [ps1.tile([1, 512], F32, tag=f"sq{sc}", bufs=1,
                                     name=f"sqps{sc}")
                            for sc in range(NSC)]
                    for a in range(NDT):
                        nc.sync.dma_start(xb[:, a, :], xb_t[:, a, :])
                        sqt = p1.tile([128, S], BF16, tag="sq", bufs=3)
                        nc.scalar.activation(sqt[:], xb[:, a, :], AF.Square)
                        for sc in range(NSC):
                            nc.tensor.matmul(sqps[sc][:], ones_b[:],
                                             sqt[:, bass.ts(sc, 512)],
                                             start=(a == 0),
                                             stop=(a == NDT - 1))
                    for sc in range(NSC):
                        nc.scalar.activation(r1row[:, bass.ts(sc, 512)],
                                             sqps[sc][:], AF.Sqrt,
                                             bias=eps1[:], scale=1.0 / D)
                        nc.vector.reciprocal(r1row[:, bass.ts(sc, 512)],
                                             r1row[:, bass.ts(sc, 512)])
                    # r1 broadcast [128,S] (f32, psum chunks) folded into
                    # the 4 rope tables in place
                    for sc in range(NSC):
                        bcp = ps1.tile([128, 512], F32, tag="bc", bufs=2)
                        nc.tensor.matmul(bcp[:], ones_r[:],
                                         r1row[:, bass.ts(sc, 512)])
                        for tb in (qcos, qsin, kcos, ksin):
                            nc.vector.tensor_tensor(
                                tb[:, bass.ts(sc, 512)],
                                tb[:, bass.ts(sc, 512)], bcp[:],
                                op=AluOpType.mult)
                    # r1col [128, NDT]: token-major r1 via DRAM roundtrip
                    nc.sync.dma_start(d_r1s.ap(), r1row[:])
                    nc.sync.dma_start(
                        r1col[:], d_r1s.ap().rearrange("o (a p) -> p (o a)",
                                                       p=128))

                # ---- phase 2: QKV + rope ----
                with tc.tile_pool(name="qkv", bufs=1) as qkp:
                    if phases >= 2:
                        ps2cm = tc.tile_pool(name="ps2", bufs=1,
                                             space="PSUM")
                        ps2 = ps2cm.__enter__()
                        wqkv = qkp.tile([128, 3, NDT, DCC], BF16, name="wqkv")
                        nc.sync.dma_start(wqkv[:, 0], wq_t)
                        nc.sync.dma_start(wqkv[:, 1], wk_t)
                        nc.sync.dma_start(wqkv[:, 2], wv_t)
                        q_sb = qkp.tile([128, HPC, S], BF16, name="q_sb")
                        k_sb = qkp.tile([128, HPC, S], BF16, name="k_sb")
                        qs_sb = qkp.tile([128, HPC, S], BF16, name="qs_sb")
                        ks_sb = qkp.tile([128, HPC, S], BF16, name="ks_sb")
                        for wi, t_sb in ((0, q_sb), (1, k_sb)):
                            for mc in range(HPC):
                                for sc in range(NSC):
                                    ps = ps2.tile([128, 512], F32, tag="mm",
                                                  bufs=4)
                                    for a in range(NDT):
                                        nc.tensor.matmul(
                                            ps[:],
                                            wqkv[:, wi, a, bass.ts(mc, 128)],
                                            xb[:, a, bass.ts(sc, 512)],
                                            start=(a == 0),
                                            stop=(a == NDT - 1))
                                    nc.scalar.copy(
                                        t_sb[:, mc, bass.ts(sc, 512)],
                                        ps[:])
                        for src, dst in ((q_sb, qs_sb), (k_sb, ks_sb)):
                            for mc in range(HPC):
                                nc.sync.dma_start(dst[0:64, mc, :],
                                                  src[64:128, mc, :])
                                nc.sync.dma_start(dst[64:128, mc, :],
                                                  src[0:64, mc, :])
                        qr = qkp.tile([128, HPC, S], BF16, name="qr")
                        kr = qkp.tile([128, HPC, S], BF16, name="kr")
                        for mc in range(HPC):
                            tq = qkp.tile([128, S], BF16, tag="rt", bufs=2)
                            nc.vector.tensor_tensor(tq[:], qs_sb[:, mc, :],
                                                    qsin[:],
                                                    op=AluOpType.mult)
                            nc.vector.tensor_tensor(qr[:, mc, :],
                                                    q_sb[:, mc, :], qcos[:],
                                                    op=AluOpType.mult)
                            nc.vector.tensor_tensor(qr[:, mc, :],
                                                    qr[:, mc, :], tq[:],
                                                    op=AluOpType.add)
                            tk = qkp.tile([128, S], BF16, tag="rt", bufs=2)
                            nc.vector.tensor_tensor(tk[:], ks_sb[:, mc, :],
                                                    ksin[:],
                                                    op=AluOpType.mult)
                            nc.vector.tensor_tensor(kr[:, mc, :],
                                                    k_sb[:, mc, :], kcos[:],
                                                    op=AluOpType.mult)
                            nc.vector.tensor_tensor(kr[:, mc, :],
                                                    kr[:, mc, :], tk[:],
                                                    op=AluOpType.add)
                        v_sb = qkp.tile([128, NDT, DCC], BF16, name="v_sb")
                        for mc in range(NDT):
                            ps = ps2.tile([128, DCC], F32, tag="mmv",
                                          bufs=3)
                            for a in range(NDT):
                                nc.tensor.matmul(
                                    ps[:], xb[:, a, bass.ts(mc, 128)],
                                    wqkv[:, 2, a, :],
                                    start=(a == 0), stop=(a == NDT - 1))
                            nc.vector.tensor_scalar(
                                v_sb[:, mc, :], ps[:],
                                r1col[:, mc:mc + 1], None,
                                op0=AluOpType.mult)
                        ps2cm.__exit__(None, None, None)

                    # ---- phase 3: attention ----
                    if phases >= 3:
                        ps3cm = tc.tile_pool(name="ps3", bufs=1,
                                             space="PSUM")
                        ps3 = ps3cm.__enter__()
                        ctxT = qkp.tile([128, HPC, S], BF16, name="ctxT")
                        if os.environ.get("DUMMY_ATTN"):
                            for h in range(HPC):
                                nc.scalar.copy(ctxT[:, h, :], qr[:, h, :])
                        else:
                            _full_attn = True
                        for h in range(HPC if not os.environ.get("DUMMY_ATTN")
                                       else 0):
                            for qc in range(NSC):
                                nkt = 4 * (qc + 1)
                                cps = ps3.tile([128, 512], F32, tag="ctx",
                                               bufs=2)
                                dps = ps3.tile([1, 512], F32, tag="den",
                                               bufs=2)
                                for kt in range(nkt):
                                    sps = ps3.tile([128, 512], F32,
                                                   tag="st", bufs=3)
                                    nc.tensor.matmul(
                                        sps[:], kr[:, h, bass.ts(kt, 128)],
                                        qr[:, h, bass.ts(qc, 512)])
                                    est = qkp.tile([128, 512], BF16,
                                                   tag="est", bufs=4)
                                    nc.scalar.activation(est[:], sps[:],
                                                         AF.Exp)
                                    if kt // 4 == qc:
                                        nc.vector.tensor_tensor(
                                            est[:], est[:], tri[:, kt % 4, :],
                                            op=AluOpType.mult)
                                    nc.tensor.matmul(
                                        cps[:], v_sb[:, kt, bass.ts(h, 128)],
                                        est[:], start=(kt == 0),
                                        stop=(kt == nkt - 1))
                                    nc.tensor.matmul(
                                        dps[:], ones_b[:], est[:],
                                        start=(kt == 0),
                                        stop=(kt == nkt - 1))
                                rrow = qkp.tile([1, 512], F32, tag="rr",
                                                bufs=2)
                                nc.vector.reciprocal(rrow[:], dps[:])
                                rbs = qkp.tile([128, 512], F32, tag="rbs",
                                               bufs=2)
                                nc.gpsimd.partition_broadcast(rbs[:],
                                                              rrow[:])
                                nc.vector.tensor_tensor(
                                    ctxT[:, h, bass.ts(qc, 512)], cps[:],
                                    rbs[:], op=AluOpType.mult)
                        for mc in range(HPC):
                            nc.sync.dma_start(cc1i_t[:, mc, :],
                                              ctxT[:, mc, :])
                        ps3cm.__exit__(None, None, None)

            # ---- phase 4: AG ctx + Wo + hs2 + transpose + gather ----
            if phases >= 4 and not os.environ.get("NO_CC"):
                if os.environ.get("TINY_CC"):
                    nc.gpsimd.collective_compute(
                        "AllGather", AluOpType.bypass, replica_groups=rg,
                        ins=[tc1_in.ap()], outs=[tc1_out.ap()])
                else:
                    nc.gpsimd.collective_compute(
                        "AllGather", AluOpType.bypass, replica_groups=rg,
                        ins=[cc1_in.ap()], outs=[cc1_out.ap()])
            with tc.tile_pool(name="p4", bufs=1) as p4:
                if phases >= 4:
                    hs2f = p4.tile([128, 2, S], F32, name="hs2f")
                    hs2tok = p4.tile([128, NDT, DCC], BF16, name="hs2tok")
                    ps4acm = tc.tile_pool(name="ps4a", bufs=1,
                                          space="PSUM")
                    ps4a = ps4acm.__enter__()
                    wopcm = tc.tile_pool(name="wo_ph", bufs=1)
                    wop = wopcm.__enter__()
                    gidxb = wop.tile([128, s2], F32, name="gidxb")
                    nc.sync.dma_start(gidxb[:], d_gidxb.ap())
                    ctxg = wop.tile([128, NDT, S], BF16, name="ctxg")
                    for a in range(NDT):
                        nc.sync.dma_start(ctxg[:, a, :], cc1o_t[:, a, :])
                    wo = wop.tile([128, NDT, DCC], BF16, name="wo")
                    nc.sync.dma_start(wo[:], wo_t)
                    wops = [ps4a.tile([128, 512], F32, tag=f"wo{mc}{sc}",
                                      bufs=1, name=f"wops{mc}{sc}")
                            for mc in range(HPC) for sc in range(NSC)]
                    for a in range(NDT):
                        for mc in range(HPC):
                            for sc in range(NSC):
                                nc.tensor.matmul(
                                    wops[mc * NSC + sc][:],
                                    wo[:, a, bass.ts(mc, 128)],
                                    ctxg[:, a, bass.ts(sc, 512)],
                                    start=(a == 0), stop=(a == NDT - 1))
                    for mc in range(HPC):
                        for sc in range(NSC):
                            ps = wops[mc * NSC + sc]
                            t = wop.tile([128, 512], F32, tag="wot",
                                         bufs=2)
                            nc.vector.tensor_tensor(
                                t[:], ps[:], ma_b[:, bass.ts(sc, 512)],
                                op=AluOpType.mult)
                            nc.vector.tensor_tensor(
                                hs2f[:, mc, bass.ts(sc, 512)], t[:],
                                hresb[:, mc, bass.ts(sc, 512)],
                                op=AluOpType.add)
                    ps4acm.__exit__(None, None, None)
                    ps4cm = tc.tile_pool(name="ps4", bufs=1, space="PSUM")
                    ps4 = ps4cm.__enter__()
                    # hs2 token-major (bf16) via PE transposes
                    hs2b = wop.tile([128, 2, S], BF16, name="hs2b")
                    for mc in range(HPC):
                        nc.scalar.copy(hs2b[:, mc, :], hs2f[:, mc, :])
                    if not os.environ.get("DUMMY_GATHER"):
                        for dc in range(HPC):
                            for tcix in range(NDT):
                                tps = ps4.tile([128, 128], BF16, tag="tp",
                                               bufs=2)
                                nc.tensor.transpose(
                                    tps[:], hs2b[:, dc, bass.ts(tcix, 128)],
                                    ident[:])
                                nc.scalar.copy(
                                    hs2tok[:, tcix, bass.ts(dc, 128)],
                                    tps[:])
                    # compact via selection matmuls:
                    # xc[d, j] = sum_t hs2tok[t, d] * (gidxb[j] == t)
                    xc = wop.tile([128, 2, s2], BF16, name="xc")
                    if os.environ.get("DUMMY_GATHER"):
                        for mc in range(HPC):
                            nc.scalar.copy(xc[:, mc, :],
                                           hs2b[:, mc, 0:s2])
                    for off, sz in ([] if os.environ.get("DUMMY_GATHER")
                                    else _chunks(s2)):
                        ps0 = ps4.tile([128, 512], F32, tag="gx0", bufs=1)
                        ps1 = ps4.tile([128, 512], F32, tag="gx1", bufs=1)
                        for tt in range(NDT):
                            sel = wop.tile([128, 512], BF16, tag="sel",
                                           bufs=3)
                            nc.vector.tensor_scalar(
                                sel[:, 0:sz], gidxb[:, off:off + sz],
                                rowpos[:, tt:tt + 1], None,
                                op0=AluOpType.is_equal)
                            nc.tensor.matmul(
                                ps0[:, 0:sz], hs2tok[:, tt, 0:128],
                                sel[:, 0:sz], start=(tt == 0),
                                stop=(tt == NDT - 1))
                            nc.tensor.matmul(
                                ps1[:, 0:sz], hs2tok[:, tt, 128:256],
                                sel[:, 0:sz], start=(tt == 0),
                                stop=(tt == NDT - 1))
                        nc.scalar.copy(xc[:, 0, off:off + sz], ps0[:, 0:sz])
                        nc.scalar.copy(xc[:, 1, off:off + sz], ps1[:, 0:sz])
                    for mc in range(HPC):
                        nc.sync.dma_start(cc2i_t[:, mc, :], xc[:, mc, :])
                    wopcm.__exit__(None, None, None)
                    ps4cm.__exit__(None, None, None)
                if phases >= 5:
                    if not os.environ.get("NO_CC"):
                        if os.environ.get("TINY_CC"):
                            nc.gpsimd.collective_compute(
                                "AllGather", AluOpType.bypass,
                                replica_groups=rg,
                                ins=[tc2_in.ap()], outs=[tc2_out.ap()])
                        else:
                            nc.gpsimd.collective_compute(
                                "AllGather", AluOpType.bypass,
                                replica_groups=rg,
                                ins=[cc2_in.ap()], outs=[cc2_out.ap()])

                    # ---- phase 5: norm2 + MLP on compact tokens ----
                    with (
                        tc.tile_pool(name="mlp", bufs=1) as mlp,
                        tc.tile_pool(name="ps5", bufs=1,
                                     space="PSUM") as ps5,
                    ):
                        hs2g = mlp.tile([128, NDT, s2], BF16, name="hs2g")
                        for a in range(NDT):
                            nc.sync.dma_start(hs2g[:, a, :], cc2o_t[:, a, :])
                        with tc.tile_pool(name="r2p", bufs=1) as r2p:
                            r2row = r2p.tile([1, s2], F32, name="r2row")
                            r2b = r2p.tile([128, s2], F32, name="r2b")
                            for sc, (off, sz) in enumerate(_chunks(s2)):
                                ssp = ps5.tile([1, 512], F32, tag="ss",
                                               bufs=2)
                                for a in range(NDT):
                                    sqt = r2p.tile([128, 512], BF16,
                                                   tag="sq2", bufs=3)
                                    nc.scalar.activation(
                                        sqt[:, 0:sz],
                                        hs2g[:, a, off:off + sz], AF.Square)
                                    nc.tensor.matmul(ssp[:, 0:sz], ones_b[:],
                                                     sqt[:, 0:sz],
                                                     start=(a == 0),
                                                     stop=(a == NDT - 1))
                                nc.scalar.activation(r2row[:, off:off + sz],
                                                     ssp[:, 0:sz], AF.Sqrt,
                                                     bias=eps1[:],
                                                     scale=1.0 / D)
                                nc.vector.reciprocal(r2row[:, off:off + sz],
                                                     r2row[:, off:off + sz])
                                bcp = ps5.tile([128, 512], F32, tag="bc2",
                                               bufs=1)
                                nc.tensor.matmul(bcp[:, 0:sz], ones_r[:],
                                                 r2row[:, off:off + sz])
                                nc.vector.tensor_copy(r2b[:, off:off + sz],
                                                      bcp[:, 0:sz])
                            for a in range(NDT):
                                nc.vector.tensor_tensor(
                                    hs2g[:, a, :], hs2g[:, a, :], r2b[:],
                                    op=AluOpType.mult)
                        xn2 = hs2g
                        hT = mlp.tile([128, NFT, s2], BF16, name="hT")
                        if os.environ.get("DUMMY_MLP"):
                            for fc in range(NFT):
                                nc.scalar.copy(hT[:, fc, :], xn2[:, fc, :])
                            stgd = mlp.tile([128, s2], BF16, name="stgd")
                            for mc in range(NDT):
                                nc.scalar.copy(stgd[:], hT[:, mc % NFT, :])
                                nc.sync.dma_start(cc3i_t[:, mc, :], stgd[:])
                        _skip_mlp = bool(os.environ.get("DUMMY_MLP"))
                        with tc.tile_pool(name="wstream", bufs=3) as wsp:
                          if not _skip_mlp:
                            for fc in range(NFT):
                                wgc = wsp.tile([128, NDT, 128], BF16,
                                               tag="wgc")
                                nc.sync.dma_start(
                                    wgc[:], wg_t[:, :, bass.ts(fc, 128)])
                                sg = wsp.tile([128, s2], BF16, tag="sg",
                                              bufs=2)
                                for off, sz in _chunks(s2):
                                    ps = ps5.tile([128, 512], F32, tag="mm",
                                                  bufs=4)
                                    for a in range(NDT):
                                        nc.tensor.matmul(
                                            ps[:, 0:sz], wgc[:, a, :],
                                            xn2[:, a, off:off + sz],
                                            start=(a == 0),
                                            stop=(a == NDT - 1))
                                    sgs = wsp.tile([128, 512], BF16,
                                                   tag="sgs", bufs=2)
                                    nc.scalar.activation(sgs[:, 0:sz],
                                                         ps[:, 0:sz],
                                                         AF.Sigmoid)
                                    nc.vector.tensor_tensor(
                                        sg[:, off:off + sz], ps[:, 0:sz],
                                        sgs[:, 0:sz], op=AluOpType.mult)
                                wuc = wsp.tile([128, NDT, 128], BF16,
                                               tag="wuc")
                                nc.sync.dma_start(
                                    wuc[:], wu_t[:, :, bass.ts(fc, 128)])
                                for off, sz in _chunks(s2):
                                    ps = ps5.tile([128, 512], F32, tag="mm",
                                                  bufs=4)
                                    for a in range(NDT):
                                        nc.tensor.matmul(
                                            ps[:, 0:sz], wuc[:, a, :],
                                            xn2[:, a, off:off + sz],
                                            start=(a == 0),
                                            stop=(a == NDT - 1))
                                    nc.vector.tensor_tensor(
                                        hT[:, fc, off:off + sz], ps[:, 0:sz],
                                        sg[:, off:off + sz],
                                        op=AluOpType.mult)
                            for mc in range(NDT):
                                wdc = wsp.tile([128, NFT, 128], BF16,
                                               tag="wdc")
                                nc.sync.dma_start(
                                    wdc[:], wd_t[:, :, bass.ts(mc, 128)])
                                stg = wsp.tile([128, s2], BF16,
                                               tag="stg", bufs=3)
                                for off, sz in _chunks(s2):
                                    ps = ps5.tile([128, 512], F32, tag="mm",
                                                  bufs=4)
                                    for a in range(NFT):
                                        nc.tensor.matmul(
                                            ps[:, 0:sz], wdc[:, a, :],
                                            hT[:, a, off:off + sz],
                                            start=(a == 0),
                                            stop=(a == NFT - 1))
                                    nc.scalar.copy(stg[:, off:off + sz],
                                                   ps[:, 0:sz])
                                nc.sync.dma_start(cc3i_t[:, mc, :], stg[:])
                if phases >= 6:
                    pre = tc.tile_pool(name="pre6", bufs=1)
                    prep = pre.__enter__()
                    bidxb = prep.tile([128, S], F32, name="bidxb")
                    nc.sync.dma_start(bidxb[:], d_bidxb.ap())
                    sels = prep.tile([128, ns2t, S], BF16, name="sels")
                    for ct in range(ns2t):
                        nc.vector.tensor_scalar(
                            sels[:, ct, :], bidxb[:],
                            rowpos[:, ct:ct + 1], None,
                            op0=AluOpType.is_equal)
                    if not os.environ.get("NO_CC"):
                        if os.environ.get("TINY_CC"):
                            nc.gpsimd.collective_compute(
                                "ReduceScatter", AluOpType.add,
                                replica_groups=rg,
                                ins=[tc3_in.ap()], outs=[tc3_out.ap()])
                        else:
                            nc.gpsimd.collective_compute(
                                "ReduceScatter", AluOpType.add,
                                replica_groups=rg,
                                ins=[cc3_in.ap()], outs=[cc3_out.ap()])

                    # ---- phase 6: expand + final residual ----
                    with (
                        tc.tile_pool(name="fin", bufs=1) as fin,
                        tc.tile_pool(name="ps6", bufs=1,
                                     space="PSUM") as ps6,
                    ):
                        rs = fin.tile([128, 2, s2], BF16, name="rs")
                        for mc in range(HPC):
                            nc.sync.dma_start(rs[:, mc, :], cc3o_t[:, mc, :])
                        rstok = fin.tile([128, ns2t, DCC], BF16,
                                         name="rstok")
                        if not os.environ.get("DUMMY_GATHER"):
                         for dc in range(HPC):
                            for tcix in range(ns2t):
                                tps = ps6.tile([128, 128], BF16, tag="tp2",
                                               bufs=3)
                                nc.tensor.transpose(
                                    tps[:], rs[:, dc, bass.ts(tcix, 128)],
                                    ident[:])
                                nc.scalar.copy(
                                    rstok[:, tcix, bass.ts(dc, 128)], tps[:])
                        # expand via selection matmuls fused with residual:
                        # out[d, s] = hs2f[d, s]
                        #             + sum_ct rstok[ct, d]*(bidxb[s] == ct)
                        outt = fin.tile([128, 2, S], F32, name="outt")
                        if os.environ.get("DUMMY_GATHER"):
                            for mc in range(HPC):
                                t9 = fin.tile([128, S], F32, tag="t9",
                                              bufs=2)
                                nc.vector.tensor_tensor(
                                    t9[:, 0:s2], rs[:, mc, :],
                                    hs2f[:, mc, 0:s2], op=AluOpType.add)
                                nc.vector.tensor_copy(outt[:, mc, :],
                                                      hs2f[:, mc, :])
                        for off, sz in ([] if os.environ.get("DUMMY_GATHER")
                                        else _chunks(S)):
                            ps0 = ps6.tile([128, 512], F32, tag="ex0",
                                           bufs=2)
                            ps1 = ps6.tile([128, 512], F32, tag="ex1",
                                           bufs=2)
                            for ct in range(ns2t):
                                nc.tensor.matmul(
                                    ps0[:, 0:sz], rstok[:, ct, 0:128],
                                    sels[:, ct, off:off + sz],
                                    start=(ct == 0),
                                    stop=(ct == ns2t - 1))
                                nc.tensor.matmul(
                                    ps1[:, 0:sz], rstok[:, ct, 128:256],
                                    sels[:, ct, off:off + sz],
                                    start=(ct == 0),
                                    stop=(ct == ns2t - 1))
                            nc.vector.tensor_tensor(
                                outt[:, 0, off:off + sz], ps0[:, 0:sz],
                                hs2f[:, 0, off:off + sz], op=AluOpType.add)
                            nc.vector.tensor_tensor(
                                outt[:, 1, off:off + sz], ps1[:, 0:sz],
                                hs2f[:, 1, off:off + sz], op=AluOpType.add)
                        for mc in range(HPC):
                            nc.sync.dma_start(out_t[:, mc, :], outt[:, mc, :])
                    pre.__exit__(None, None, None)

    nc.compile()
    return nc


def _rope_tables():
    pos = np.arange(S, dtype=np.float32)
    inv = 1.0 / (THETA ** (np.arange(0, Dh, 2, dtype=np.float32) / Dh))
    ang = pos[:, None] * inv[None, :]
    emb = np.concatenate([ang, ang], axis=-1)          # [S, Dh]
    cosT = np.cos(emb).T.astype(np.float32).copy()     # [Dh, S]
    ssinT = np.sin(emb).T.astype(np.float32).copy()
    ssinT[:64] = -ssinT[:64]
    return cosT, ssinT


def _tri_masks():
    m = np.zeros((128, 4, 512), np.float32)
    for i in range(4):
        j = np.arange(512)[None, :]
        p = np.arange(128)[:, None]
        m[:, i, :] = ((j - 128 * i) >= p).astype(np.float32)
        m[:, i, : 128 * i] = 0.0
        m[:, i, 128 * (i + 1):] = 1.0
    return m.reshape(128, 4 * 512)


def _wrap_idx(vals, n):
    """Index layout for dma_gather: [128, n//16] int16, idx j at partition
    j%16 (replicated across the 8 16-partition groups), free pos j//16."""
    t = np.zeros((128, n // 16), np.int16)
    v = np.asarray(vals, np.int16)
    for grp in range(8):
        t[grp * 16:(grp + 1) * 16, :] = v.reshape(n // 16, 16).T
    return t


def kernel(**inputs):
    bf = ml_dtypes.bfloat16
    hs = np.ascontiguousarray(np.asarray(inputs["hidden_states"],
                                         np.float32)[0])
    ln1 = np.asarray(inputs["ln1_w"], np.float32)
    ln2 = np.asarray(inputs["ln2_w"], np.float32)
    Wq = np.asarray(inputs["Wq"], np.float32) * ln1[:, None]
    Wk = np.asarray(inputs["Wk"], np.float32) * ln1[:, None]
    Wv = np.asarray(inputs["Wv"], np.float32) * ln1[:, None]
    Wo = np.asarray(inputs["Wo"], np.float32)
    wg = np.asarray(inputs["w_gate"], np.float32) * ln2[:, None]
    wu = np.asarray(inputs["w_up"], np.float32) * ln2[:, None]
    wd = np.asarray(inputs["w_down"], np.float32)
    raw = np.asarray(inputs["router_attn_w"], np.float64)
    rab = np.asarray(inputs["router_attn_b"], np.float64)
    rmw = np.asarray(inputs["router_mlp_w"], np.float64)
    rmb = np.asarray(inputs["router_mlp_b"], np.float64)

    # host-side routing (fp64; exact vs fp32 reference since margins >> eps)
    la = hs.astype(np.float64) @ raw + rab
    lm = hs.astype(np.float64) @ rmw + rmb
    route_attn = la[:, 1] > la[:, 0]          # True -> attn output zeroed
    route_mlp = lm[:, 1] > lm[:, 0]           # True -> mlp skipped
    ma = (~route_attn).astype(np.float32)     # 1 -> keep attn out
    mm = (~route_mlp).astype(np.float32)      # 1 -> apply mlp
    kept = np.nonzero(mm)[0]
    cnt = len(kept)
    s2 = max(128, -(-cnt // 128) * 128)
    gidx_vals = np.zeros(s2, np.int64)
    gidx_vals[:cnt] = kept
    back = np.zeros(S, np.int64)
    back[kept] = np.arange(cnt)

    hsT = np.ascontiguousarray(hs.T)                   # [D, S]
    cosT, ssinT = _rope_tables()
    sc = np.float32(1.0 / np.sqrt(Dh))
    tri = np.ascontiguousarray(_tri_masks().astype(bf))
    qcos = np.ascontiguousarray(cosT.astype(bf))
    qsin = np.ascontiguousarray(ssinT.astype(bf))
    kcos = np.ascontiguousarray((cosT * sc).astype(bf))
    ksin = np.ascontiguousarray((ssinT * sc).astype(bf))
    ma_b = np.ascontiguousarray(
        np.broadcast_to(ma[None, :], (128, S)).astype(bf))
    ident = np.eye(128, dtype=np.float32).astype(bf)
    gidxv = np.full(s2, -1.0, np.float32)
    gidxv[:cnt] = kept.astype(np.float32)
    gidxb = np.ascontiguousarray(
        np.broadcast_to(gidxv[None, :], (128, s2)).astype(np.float32))
    bidxv = np.full(S, -1.0, np.float32)
    bidxv[kept] = np.arange(cnt, dtype=np.float32)
    bidxb = np.ascontiguousarray(
        np.broadcast_to(bidxv[None, :], (128, S)).astype(np.float32))
    rowpos = np.ascontiguousarray(
        (np.arange(128, dtype=np.float32)[:, None]
         + 128.0 * np.arange(NDT, dtype=np.float32)[None, :]))

    key = ("nc", s2)
    if key not in _CACHE:
        _CACHE["nc"] = _build_program(s2)
        _CACHE["key"] = key
    nc = _CACHE["nc"]

    xb_full = np.ascontiguousarray(hsT.astype(bf))
    in_maps = []
    for c in range(NC):
        dsl = slice(c * DCC, (c + 1) * DCC)
        fsl = slice(c * FPC, (c + 1) * FPC)
        in_maps.append({
            "xb": xb_full,
            "hresb": np.ascontiguousarray(xb_full[dsl]),
            "wq": np.ascontiguousarray(Wq[:, dsl].astype(bf)),
            "wk": np.ascontiguousarray(Wk[:, dsl].astype(bf)),
            "wv": np.ascontiguousarray(Wv[:, dsl].astype(bf)),
            "wo": np.ascontiguousarray(Wo[:, dsl].astype(bf)),
            "wg": np.ascontiguousarray(wg[:, fsl].astype(bf)),
            "wu": np.ascontiguousarray(wu[:, fsl].astype(bf)),
            "wd": np.ascontiguousarray(wd[fsl].astype(bf)),
            "qcos": qcos, "qsin": qsin, "kcos": kcos, "ksin": ksin,
            "tri": tri, "ma": ma_b, "ident": ident,
            "gidxb": gidxb, "bidxb": bidxb, "rowpos": rowpos,
        })
    _CACHE["in_maps"] = in_maps
    res = run_bass_kernel_spmd(nc, in_maps, core_ids=list(range(NC)))
    _CACHE["res"] = res
    outT = np.concatenate([res.results[c]["out"] for c in range(NC)], axis=0)
    return np.ascontiguousarray(outT.T)[None]


if __name__ == "__main__":
    import reference
    inputs = reference.setup_inputs()
    out = kernel(**inputs)
    print(out.shape, out.dtype)
